# revision 20
# baseline (speedup 1.0000x reference)
"""BiLSTM-CRF forward NLL on 8 Trainium2 NeuronCores.

Sharding: pure data-parallel over batch (8 sequences per core), params
replicated. Per core: embedding gather -> bulk input matmuls -> 2-layer
BiLSTM recurrence (fwd/bwd chains interleaved per layer) -> emissions ->
CRF scan -> partial (num, denom) pair. Host sums partials with the
label-dependent numerator constant.

Key restructurings (validated against the reference):
  * LSTM cell uses a single tanh activation per step over all 4 gates:
    sigmoid(x) = (tanh(x/2)+1)/2, with the tanh(0.5*...) instruction scale
    folded into host-prescaled weights (g-gate rows x2). The cell tracks
    C = 2c and H = 2h; every consumer of h (recurrent weights, layer-1
    input weights, output projection) is pre-halved on the host.
  * Gate slots are laid out (o,i,f,g) with a per-dir persistent cell tile
    [o i f g C] so one fused DVE op computes v2=(yi+1)*yg and z=(yf+1)*C.
  * All tile pools coexist (PSUM banks budgeted to 8) so the readiness-
    driven scheduler overlaps: gather with bulk-0 (gather emitted from
    both sequence ends, bulk-0 bwd chunks reversed), bulk-1 with the
    layer-0 recurrence (h0 becomes available middle-out), and emissions
    (+exp +numerator partials) with the layer-1 recurrence.
  * CRF runs in probability space, split into a forward alpha chain
    (t=0..255) and a backward beta chain (t=511..256) that execute
    concurrently, halving the sequential CRF depth. State ping-pongs
    between PSUM (matmul out) and SBUF (elementwise mult with expem),
    with one ln-renorm per chain. em' = em + b_out - log(L); the log(L)
    shift cancels between numerator and denominator.
"""

import os
import sys

import numpy as np

sys.path.insert(0, "/opt/trn_rl_repo")

import concourse.bass as bass
import concourse.tile as tile
from concourse import bacc, mybir
from concourse.bass_utils import run_bass_kernel_spmd

B, T, V, D, HD, L = 64, 512, 100000, 300, 256, 9
H = 128
NCORES = 8
BL = B // NCORES          # sequences per core
DPAD = 384                # D padded so DMA-transpose chunks are 128 wide
KCH = (128, 128, 128)     # K chunks of DPAD
CBAR = float(np.log(L))   # per-step CRF shift (cancels in num - denom)

f32 = mybir.dt.float32
bf16 = mybir.dt.bfloat16
i32 = mybir.dt.int32
ALU = mybir.AluOpType
ACTF = mybir.ActivationFunctionType


# ---------------------------------------------------------------------------
# device program
# ---------------------------------------------------------------------------

def build_program(Tsteps=T):
    NT = Tsteps * BL
    NCK = max(1, NT // 512)            # bulk matmul N chunks
    NCOLS = NT // NCK
    NTILES = NT // 128                 # gather tiles
    assert NT % 128 == 0 and NT % NCK == 0

    nc = bacc.Bacc("TRN2", target_bir_lowering=False, debug=False)

    def din(name, shape, dt):
        return nc.dram_tensor(name, shape, dt, kind="ExternalInput").ap()

    words = din("words", [128, NTILES], i32)
    emb = din("emb", [V, DPAD], bf16)
    ident = din("ident", [128, 128], bf16)
    # lhsT weights, gate-major free dim (slots o,i,f,g each 128 wide)
    wih0 = din("wih0", [2, 3, 128, 512], bf16)     # [dir][kchunk][K][4*128]
    wih1 = din("wih1", [2, 2, 128, 512], bf16)     # [dir][h0-dir kchunk][K][4*128]
    whh = din("whh", [2, 2, 128, 512], bf16)       # [layer][dir][K=H][4*128]
    biases = din("biases", [2, 2, 128, 4], f32)    # [layer][dir][hidden][gate]
    woutT = din("woutT", [2, 128, L], bf16)        # [h1-dir kchunk][K][L]
    bout = din("bout", [L, 1], f32)                # b_out - CBAR
    estart = din("estart", [L, 1], f32)            # exp(start_t)
    expE = din("expE", [L, L], f32)                # exp(trans)
    expET = din("expET", [L, L], f32)              # exp(trans).T
    expend = din("expend", [L, 1], f32)            # exp(end_t)
    oh = din("oh", [L, NT], bf16)                  # label one-hot, (t,b) order
    res = nc.dram_tensor("res", [1, 2], f32, kind="ExternalOutput").ap()

    with tile.TileContext(nc) as tc:
        _emit(tc, nc, Tsteps, NT, NCK, NCOLS, NTILES,
              words, emb, ident, wih0, wih1, whh, biases, woutT, bout,
              estart, expE, expET, expend, oh, res)
    nc.compile()
    return nc


def _emit(tc, nc, Tsteps, NT, NCK, NCOLS, NTILES,
          words, emb, ident, wih0, wih1, whh, biases, woutT, bout,
          estart, expE, expET, expend, oh, res):
    from contextlib import ExitStack

    TM = Tsteps // 2            # alpha covers t<=TM-1... (split point)
    RENORM_K = 128              # renorm once per chain at this chain-step

    ctx = ExitStack()
    with ctx:
        consts = ctx.enter_context(tc.tile_pool(name="consts", bufs=1))
        states = ctx.enter_context(tc.tile_pool(name="states", bufs=1))
        # PSUM budget (8 banks): bulkp 2 + gates 2 + transpose 2 + alpha/beta 2
        bulkp = ctx.enter_context(
            tc.tile_pool(name="bulkp", bufs=2, space="PSUM"))
        gatesp = ctx.enter_context(
            tc.tile_pool(name="gatesp", bufs=1, space="PSUM"))
        tpp = ctx.enter_context(
            tc.tile_pool(name="tpp", bufs=2, space="PSUM"))
        crfp = ctx.enter_context(
            tc.tile_pool(name="crfp", bufs=1, space="PSUM"))
        wpool = ctx.enter_context(tc.tile_pool(name="wpool", bufs=4))
        scrp = ctx.enter_context(tc.tile_pool(name="scrp", bufs=2))
        crfs = ctx.enter_context(tc.tile_pool(name="crfs", bufs=4))
        xTp = ctx.enter_context(tc.tile_pool(name="xT", bufs=1))

        # ---- persistent SBUF tiles ----
        ident_sb = consts.tile([128, 128], bf16, tag="ident")
        nc.sync.dma_start(ident_sb[:], ident[:])
        whh_sb = {}
        for l in range(2):
            for d in range(2):
                t_ = consts.tile([128, 512], bf16, name=f"whh{l}{d}")
                nc.sync.dma_start(t_[:], whh[l, d])
                whh_sb[l, d] = t_
        bias_sb = {}
        for l in range(2):
            for d in range(2):
                t_ = consts.tile([128, 4], f32, name=f"bias{l}{d}")
                nc.sync.dma_start(t_[:], biases[l, d])
                bias_sb[l, d] = t_
        wih0_sb = {}
        for d in range(2):
            for c in range(3):
                t_ = consts.tile([128, 512], bf16, name=f"wih0_{d}{c}")
                nc.sync.dma_start(t_[:], wih0[d, c])
                wih0_sb[d, c] = t_
        wih1_sb = {}
        for d in range(2):
            for k in range(2):
                t_ = consts.tile([128, 512], bf16, name=f"wih1_{d}{k}")
                nc.sync.dma_start(t_[:], wih1[d, k])
                wih1_sb[d, k] = t_
        woutT_sb = [consts.tile([128, L], bf16, name=f"wo{k}") for k in range(2)]
        for k in range(2):
            nc.sync.dma_start(woutT_sb[k][:], woutT[k])
        bout_sb = consts.tile([L, 1], f32, name="bout_sb")
        estart_sb = consts.tile([L, 1], f32, name="estart_sb")
        expE_sb = consts.tile([L, L], f32, name="expE_sb")
        expET_sb = consts.tile([L, L], f32, name="expET_sb")
        expend_sb = consts.tile([L, 1], f32, name="expend_sb")
        ones9 = consts.tile([L, L], f32, name="ones9")
        nc.sync.dma_start(bout_sb[:], bout[:])
        nc.sync.dma_start(estart_sb[:], estart[:])
        nc.sync.dma_start(expE_sb[:], expE[:])
        nc.sync.dma_start(expET_sb[:], expET[:])
        nc.sync.dma_start(expend_sb[:], expend[:])
        nc.vector.memset(ones9[:], 1.0)

        h_hist = {}
        for l in range(2):
            for d in range(2):
                h_hist[l, d] = states.tile([128, NT], bf16, name=f"h{l}{d}")
        # per-dir persistent cell tile, cols [o i f g C] (x BL each); the
        # C slot makes the fused (v2|z) DVE op's in1 = [g C] contiguous.
        ycell = [states.tile([128, 5 * BL], f32, name=f"yc{d}") for d in range(2)]
        tcl_st = [states.tile([128, BL], f32, name=f"tcl{d}") for d in range(2)]

        # xp for the two directions of the current layer (reused across layers)
        xp_sb = [states.tile([128, 4 * NT], bf16, name=f"xp{d}") for d in range(2)]

        # CRF persistent tiles
        expem = states.tile([L, NT], f32, name="expem")
        oh_sb = states.tile([L, NT], bf16, name="oh_sb")
        nc.sync.dma_start(oh_sb[:], oh[:])
        P_a = states.tile([L, BL], f32, name="P_a")       # alpha (SBUF leg)
        u_b = states.tile([L, BL], f32, name="u_b")       # beta (SBUF leg)
        lnacc = states.tile([1, BL], f32, name="lnacc")
        num9c = states.tile([L, NCK], f32, name="num9c")
        num9 = states.tile([L, 1], f32, name="num9")
        nc.vector.memset(lnacc[:], 0.0)

        def bulk_chunk(layer, d, nck, srcs):
            """xp[d][:, chunk nck] = srcs-matmul + bias, gate slots (o,i,f,g)."""
            nsl = slice(nck * NCOLS, (nck + 1) * NCOLS)
            xv = xp_sb[d][:].rearrange("p (t g b) -> p t g b", g=4, b=BL)
            tpc = NCOLS // BL
            for slot in range(4):
                pt = bulkp.tile([128, NCOLS], f32, tag="pt", name="pt")
                for ki, (src, wt, kk) in enumerate(srcs):
                    nc.tensor.matmul(
                        pt[:],
                        lhsT=wt[:kk, slot * 128:(slot + 1) * 128],
                        rhs=src[:kk, nsl],
                        start=(ki == 0), stop=(ki == len(srcs) - 1),
                    )
                nc.vector.tensor_scalar(
                    out=xv[:, nck * tpc:(nck + 1) * tpc, slot, :],
                    in0=pt[:].rearrange("p (t b) -> p t b", b=BL),
                    scalar1=bias_sb[layer, d][:, slot:slot + 1],
                    scalar2=None, op0=ALU.add,
                )

        def recur_phase(layer):
            # Two independent dir-chains; each cell is PE -> ACT -> DVE(w)
            # -> DVE(C) -> ACT -> DVE(h). Wall time = T x chain path; the
            # two chains overlap on the engines.
            for d in range(2):
                nc.vector.memset(ycell[d][:, 4 * BL:5 * BL], 0.0)
            for t in range(Tsteps):
                taus = (t, Tsteps - 1 - t)
                first = (t == 0)
                for d in range(2):
                    tau = taus[d]
                    y = ycell[d]
                    gp = gatesp.tile([128, 4 * BL], f32, tag=f"gp{d}",
                                     name=f"gp{d}")
                    nc.tensor.matmul(gp[:], lhsT=ident_sb[:],
                                     rhs=xp_sb[d][:, tau * 4 * BL:(tau + 1) * 4 * BL],
                                     start=True, stop=first)
                    if not first:
                        prev = tau - 1 if d == 0 else tau + 1
                        hh = h_hist[layer, d]
                        whh_t = whh_sb[layer, d]
                        for slot in range(4):
                            nc.tensor.matmul(
                                gp[:, slot * BL:(slot + 1) * BL],
                                lhsT=whh_t[:, slot * 128:(slot + 1) * 128],
                                rhs=hh[:, prev * BL:(prev + 1) * BL],
                                start=False, stop=(slot == 3))
                    # y[0:4BL] = tanh(gates/2), slots (o,i,f,g)
                    nc.scalar.activation(y[:, 0:4 * BL], gp[:], ACTF.Tanh,
                                         scale=0.5)
                    # w = [(yi+1)*yg | (yf+1)*C_old] = [v2 | z]
                    w = wpool.tile([128, 2 * BL], f32, tag=f"w{d}", name=f"w{d}")
                    nc.vector.scalar_tensor_tensor(
                        w[:], in0=y[:, BL:3 * BL], scalar=1.0,
                        in1=y[:, 3 * BL:5 * BL], op0=ALU.add, op1=ALU.mult)
                    # C = 0.5*z + v2
                    nc.vector.scalar_tensor_tensor(
                        y[:, 4 * BL:5 * BL], in0=w[:, BL:2 * BL], scalar=0.5,
                        in1=w[:, 0:BL], op0=ALU.mult, op1=ALU.add)
                    nc.scalar.activation(tcl_st[d][:], y[:, 4 * BL:5 * BL],
                                         ACTF.Tanh, scale=0.5)
                    nc.vector.scalar_tensor_tensor(
                        h_hist[layer, d][:, tau * BL:(tau + 1) * BL],
                        in0=y[:, 0:BL], scalar=1.0, in1=tcl_st[d][:],
                        op0=ALU.add, op1=ALU.mult)

        # =================================================================
        # Phase 1: embedding gather + transpose (emitted from both sequence
        # ends so both bulk-0 dir-chunk streams start early)
        # =================================================================
        x_T = [xTp.tile([k, NT], bf16, name=f"xT{c}") for c, k in enumerate(KCH)]
        idx_all = consts.tile([128, NTILES], i32, name="idx_all")
        nc.sync.dma_start(idx_all[:], words[:])
        with tc.tile_pool(name="gath", bufs=4) as gp_:
            order = []
            lo, hi = 0, NTILES - 1
            while lo <= hi:
                order.append(lo)
                if hi != lo:
                    order.append(hi)
                lo, hi = lo + 1, hi - 1
            for i in order:
                g = gp_.tile([128, DPAD], bf16, tag="g", name="g")
                nc.gpsimd.indirect_dma_start(
                    out=g[:], out_offset=None, in_=emb[:],
                    in_offset=bass.IndirectOffsetOnAxis(ap=idx_all[:, i:i + 1],
                                                        axis=0),
                )
                # transpose on the (idle) PE + DVE copy-back: keeps the
                # HWDGE free so it never stalls the frozen PE stream.
                for c, k in enumerate(KCH):
                    tp = tpp.tile([128, 128], bf16, tag="tp", name="tp")
                    nc.tensor.transpose(tp[:], g[:, c * 128:(c + 1) * 128],
                                        ident_sb[:])
                    nc.vector.tensor_copy(out=x_T[c][:, i * 128:(i + 1) * 128],
                                          in_=tp[:])

        # bulk-0: fwd chunks ascending, bwd chunks descending, interleaved.
        # Deprioritized so the scheduler runs recurrence ops first in any
        # engine-idle gap (bulk fills the slack; data deps still hold).
        LOW = -10_000_000
        srcs0 = lambda d: [(x_T[c], wih0_sb[d, c], KCH[c]) for c in range(3)]
        with tc.high_priority(offset=LOW):
            for j in range(NCK):
                bulk_chunk(0, 0, j, srcs0(0))
                bulk_chunk(0, 1, NCK - 1 - j, srcs0(1))

        recur_phase(0)

        # bulk-1 middle-out: h0 regions complete middle-out during recur 0,
        # so these run concurrently with the tail of the layer-0 recurrence.
        srcs1 = lambda d: [(h_hist[0, k], wih1_sb[d, k], 128) for k in range(2)]
        mid_order = []
        lo, hi = NCK // 2 - 1, NCK // 2
        while lo >= 0:
            mid_order.extend([hi, lo])
            lo, hi = lo - 1, hi + 1
        with tc.high_priority(offset=LOW):
            for j in mid_order:
                for d in range(2):
                    bulk_chunk(1, d, j, srcs1(d))

        recur_phase(1)

        # =================================================================
        # Emissions (middle-out, overlap recur 1): per chunk
        #   pt = w_out @ h1 (PSUM); expem = exp(pt + bout') (ACT);
        #   num partial = sum(pt * onehot) via accum_out (DVE)
        # =================================================================
        with tc.high_priority(offset=LOW):
            for nck in mid_order:
                nsl = slice(nck * NCOLS, (nck + 1) * NCOLS)
                pt = bulkp.tile([L, NCOLS], f32, tag="pt", name="pt")
                for k in range(2):
                    nc.tensor.matmul(pt[:], lhsT=woutT_sb[k][:],
                                     rhs=h_hist[1, k][:, nsl],
                                     start=(k == 0), stop=(k == 1))
                nc.scalar.activation(expem[:, nsl], pt[:], ACTF.Exp,
                                     bias=bout_sb[:, 0:1])
                scr = scrp.tile([L, NCOLS], f32, tag="scr", name="scr")
                nc.vector.scalar_tensor_tensor(
                    scr[:], in0=pt[:], scalar=0.0, in1=oh_sb[:, nsl],
                    op0=ALU.add, op1=ALU.mult,
                    accum_out=num9c[:, nck:nck + 1])

        # =================================================================
        # CRF: two-ended scan in probability space.
        #   alpha: P_{t} = (E^T P_{t-1}) o expem_t   for t = 1..TM-1
        #   beta:  Q_{t} = E (expem_{t+1} o Q_{t+1}) for t = T-2..TM-1
        #   Z_b = sum_i alpha_{TM-1}[i] * beta_{TM-1}[i]
        # State ping-pongs PSUM (matmul) <-> SBUF (mult); one renorm each.
        # =================================================================
        def renorm(vec_sb, b_lnacc_col):
            srow = bulkp.tile([L, BL], f32, tag="pt", name="srow")
            nc.tensor.matmul(srow[:], lhsT=ones9[:], rhs=vec_sb[:],
                             start=True, stop=True)
            lns = crfs.tile([1, BL], f32, tag="lns", name="lns")
            nc.scalar.activation(lns[:], srow[0:1, :], ACTF.Ln)
            nc.vector.tensor_tensor(out=lnacc[:], in0=lnacc[:], in1=lns[:],
                                    op=ALU.add)
            rec = crfs.tile([L, BL], f32, tag="rec", name="rec")
            nc.vector.reciprocal(rec[:], srow[:])
            nc.vector.tensor_tensor(out=vec_sb[:], in0=vec_sb[:], in1=rec[:],
                                    op=ALU.mult)

        # alpha init: P_0 = exp(start) o expem_0
        nc.vector.tensor_scalar(out=P_a[:], in0=expem[:, 0:BL],
                                scalar1=estart_sb[:, 0:1], scalar2=None,
                                op0=ALU.mult)
        # beta init: u = expem_{T-1} o expend  (u for producing Q_{T-2})
        nc.vector.tensor_scalar(out=u_b[:], in0=expem[:, (Tsteps - 1) * BL:],
                                scalar1=expend_sb[:, 0:1], scalar2=None,
                                op0=ALU.mult)
        # chains: alpha consumes expem_1..expem_{TM-1};
        # beta consumes expem_{T-2}..expem_{TM} then combines at TM-1.
        n_a = TM - 1                 # alpha steps (t = 1..TM-1)
        n_b = Tsteps - TM            # beta matmul steps producing Q_{TM-1}
        for k in range(1, max(n_a, n_b) + 1):
            if k <= n_a:
                sp = crfp.tile([L, BL], f32, tag="sp", name="sp")
                nc.tensor.matmul(sp[:], lhsT=expE_sb[:], rhs=P_a[:],
                                 start=True, stop=True)
                nc.gpsimd.tensor_tensor(
                    out=P_a[:], in0=sp[:],
                    in1=expem[:, k * BL:(k + 1) * BL], op=ALU.mult)
                if k == RENORM_K:
                    renorm(P_a, 0)
            if k <= n_b:
                sq = crfp.tile([L, BL], f32, tag="sq", name="sq")
                nc.tensor.matmul(sq[:], lhsT=expET_sb[:], rhs=u_b[:],
                                 start=True, stop=True)
                t_next = Tsteps - 1 - k      # Q_{t_next} now in sq
                if k < n_b:
                    nc.vector.tensor_tensor(
                        out=u_b[:], in0=sq[:],
                        in1=expem[:, t_next * BL:(t_next + 1) * BL],
                        op=ALU.mult)
                    if k == RENORM_K:
                        renorm(u_b, 0)
                else:
                    # final: u_b <- Q_{TM-1} (plain copy out of PSUM)
                    nc.vector.tensor_copy(out=u_b[:], in_=sq[:])

        # combine: Z_b = sum_i P_a[i,b] * u_b[i,b]; denom = ln Z + lnacc
        ab = crfs.tile([L, BL], f32, tag="ab", name="ab")
        nc.vector.tensor_tensor(out=ab[:], in0=P_a[:], in1=u_b[:], op=ALU.mult)
        zrow = bulkp.tile([L, BL], f32, tag="pt", name="zrow")
        nc.tensor.matmul(zrow[0:1, :], lhsT=ones9[:, 0:1], rhs=ab[:],
                         start=True, stop=True)
        lnz = crfs.tile([1, BL], f32, tag="lns", name="lnz")
        nc.scalar.activation(lnz[:], zrow[0:1, :], ACTF.Ln)
        nc.vector.tensor_tensor(out=lnz[:], in0=lnz[:], in1=lnacc[:],
                                op=ALU.add)
        dsc = crfs.tile([1, 1], f32, tag="dsc", name="dsc")
        nc.vector.tensor_reduce(dsc[:], lnz[:], axis=mybir.AxisListType.X,
                                op=ALU.add)
        nc.vector.tensor_reduce(num9[:], num9c[:], axis=mybir.AxisListType.X,
                                op=ALU.add)
        npsum = bulkp.tile([L, BL], f32, tag="pt", name="npsum")
        nc.tensor.matmul(npsum[0:1, 0:1], lhsT=ones9[:, 0:1], rhs=num9[:, 0:1],
                         start=True, stop=True)
        out_sb = crfs.tile([1, 2], f32, tag="out_sb", name="out_sb")
        nc.vector.tensor_scalar(out=out_sb[:, 0:1], in0=npsum[0:1, 0:1],
                                scalar1=0.0, scalar2=None, op0=ALU.add)
        nc.vector.tensor_scalar(out=out_sb[:, 1:2], in0=dsc[:],
                                scalar1=0.0, scalar2=None, op0=ALU.add)
        nc.sync.dma_start(res[:], out_sb[:])


# ---------------------------------------------------------------------------
# host side
# ---------------------------------------------------------------------------

def _prescale(w_ih, w_hh, b_ih, b_hh, h_in_doubled):
    """Gate-slot layout is torch order (i,f,g,o). Returns fp32 arrays."""
    sg = np.ones((4, 1), np.float32)
    sg[2] = 2.0                       # g gate rows x2 (tanh scale 0.5 trick)
    srows = np.repeat(sg, H, axis=0)  # [512, 1]
    wih = w_ih.astype(np.float32) * srows
    whh_ = w_hh.astype(np.float32) * srows * 0.5
    b = (b_ih + b_hh).astype(np.float32) * srows[:, 0]
    if h_in_doubled:
        wih = wih * 0.5
    return wih, whh_, b


GATE_ORDER = (3, 0, 1, 2)   # device slot s holds torch gate GATE_ORDER[s]: o,i,f,g


def _lhsT_gate_major(w, kchunks):
    """w: [4H, K] fp32 -> [nchunk, 128, 512] bf16 lhsT (zero-padded K)."""
    outs = []
    off = 0
    for kk in kchunks:
        blk = np.zeros((128, 512), np.float32)
        take = min(kk, w.shape[1] - off)
        for slot, g in enumerate(GATE_ORDER):
            blk[:take, slot * 128:(slot + 1) * 128] = \
                w[g * H:(g + 1) * H, off:off + take].T
        outs.append(blk)
        off += kk
    return np.stack(outs).astype(np.dtype("bfloat16"))


_PROG_CACHE = {}


def _get_program(Tsteps):
    if Tsteps not in _PROG_CACHE:
        _PROG_CACHE[Tsteps] = build_program(Tsteps)
    return _PROG_CACHE[Tsteps]


def prepare_inputs(inputs, Tsteps=T):
    """Build the per-core input maps + the host numerator constants."""
    bfl = np.dtype("bfloat16")
    words = np.asarray(inputs["word_batch"]).astype(np.int64)
    labels = np.asarray(inputs["label_batch"]).astype(np.int64)
    emb = np.asarray(inputs["emb"], np.float32)
    words = words[:, :Tsteps]
    labels = labels[:, :Tsteps]

    embp = np.zeros((V, DPAD), np.float32)
    embp[:, :D] = emb
    embp = embp.astype(bfl)

    ident = np.eye(128, dtype=np.float32).astype(bfl)

    wih0_l, whh_l, wih1_l, bias_l = [], [], [], []
    for layer, (wihk, whhk, bihk, bhhk) in enumerate(
            [("w_ih_l0", "w_hh_l0", "b_ih_l0", "b_hh_l0"),
             ("w_ih_l1", "w_hh_l1", "b_ih_l1", "b_hh_l1")]):
        for d in range(2):
            wih, whh_, b = _prescale(
                np.asarray(inputs[wihk])[d], np.asarray(inputs[whhk])[d],
                np.asarray(inputs[bihk])[d], np.asarray(inputs[bhhk])[d],
                h_in_doubled=(layer == 1))
            if layer == 0:
                wihp = np.zeros((512, DPAD), np.float32)
                wihp[:, :D] = wih
                wih0_l.append(_lhsT_gate_major(wihp, KCH))
            else:
                wih1_l.append(_lhsT_gate_major(wih, (128, 128)))
            whh_l.append(_lhsT_gate_major(whh_, (128,)))
            bias_l.append(b.reshape(4, H)[list(GATE_ORDER)].T)  # [128, 4]
    wih0 = np.stack(wih0_l)                       # [2, 3, 128, 512]
    wih1 = np.stack(wih1_l)                       # [2, 2, 128, 512]
    whh = np.stack(whh_l).reshape(2, 2, 1, 128, 512)[:, :, 0]
    biases = np.stack(bias_l).reshape(2, 2, 128, 4).astype(np.float32)

    w_out = np.asarray(inputs["w_out"], np.float32) * 0.5   # [L, 2H]
    woutT = np.stack([w_out[:, :H].T, w_out[:, H:].T]).astype(bfl)  # [2,128,L]
    boutp = (np.asarray(inputs["b_out"], np.float32) - CBAR).reshape(L, 1)
    estart = np.exp(np.asarray(inputs["start_t"], np.float32)).reshape(L, 1)
    expEm = np.exp(np.asarray(inputs["trans"], np.float32))
    expend = np.exp(np.asarray(inputs["end_t"], np.float32)).reshape(L, 1)

    NT = Tsteps * BL
    in_maps = []
    num_consts = []
    start_t = np.asarray(inputs["start_t"], np.float32)
    end_t = np.asarray(inputs["end_t"], np.float32)
    trans = np.asarray(inputs["trans"], np.float32)
    bshift = np.asarray(inputs["b_out"], np.float32) - CBAR   # device em offset
    for c in range(NCORES):
        bs = slice(c * BL, (c + 1) * BL)
        wc = words[bs]                        # [BL, Tsteps]
        lc = labels[bs]
        toks = wc.T.reshape(-1).astype(np.int32)          # (t, b) order
        ohc = (lc.T.reshape(1, -1) == np.arange(L).reshape(L, 1))
        in_maps.append({
            "words": np.ascontiguousarray(toks.reshape(NT // 128, 128).T),
            "emb": embp, "ident": ident,
            "wih0": wih0, "wih1": wih1, "whh": whh, "biases": biases,
            "woutT": woutT, "bout": boutp, "estart": estart,
            "expE": expEm, "expET": np.ascontiguousarray(expEm.T),
            "expend": expend,
            "oh": ohc.astype(bfl),
        })
        num_consts.append(
            float(start_t[lc[:, 0]].sum())
            + float(trans[lc[:, :-1], lc[:, 1:]].sum())
            + float(end_t[lc[:, -1]].sum())
            + float(bshift[lc].sum()))        # device num omits the em bias
    return in_maps, num_consts


def kernel(**inputs):
    in_maps, num_consts = prepare_inputs(inputs, T)
    nc = _get_program(T)
    out = run_bass_kernel_spmd(nc, in_maps, list(range(NCORES)))
    llh = 0.0
    for c in range(NCORES):
        r = out.results[c]["res"].reshape(2).astype(np.float64)
        llh += num_consts[c] + r[0] - r[1]
    return np.float32(-llh)


if __name__ == "__main__":
    np.random.seed(0)
    print("building program (small T) ...")
    build_program(16)
    print("ok")


# revision 22
# speedup vs baseline: 1.0095x; 1.0095x over previous
"""BiLSTM-CRF forward NLL on 8 Trainium2 NeuronCores.

Sharding: pure data-parallel over batch (8 sequences per core), params
replicated. Per core: embedding gather -> bulk input matmuls -> 2-layer
BiLSTM recurrence (fwd/bwd chains interleaved per layer) -> emissions ->
CRF scan -> partial (num, denom) pair. Host sums partials with the
label-dependent numerator constant.

Key restructurings (validated against the reference):
  * LSTM cell uses a single tanh activation per step over all 4 gates:
    sigmoid(x) = (tanh(x/2)+1)/2, with the tanh(0.5*...) instruction scale
    folded into host-prescaled weights (g-gate rows x2). The cell tracks
    C = 2c and H = 2h; every consumer of h (recurrent weights, layer-1
    input weights, output projection) is pre-halved on the host.
  * Gate slots are laid out (o,i,f,g) with a per-dir persistent cell tile
    [o i f g C] so one fused DVE op computes v2=(yi+1)*yg and z=(yf+1)*C.
  * All tile pools coexist (PSUM banks budgeted to 8) so the readiness-
    driven scheduler overlaps: gather with bulk-0 (gather emitted from
    both sequence ends, bulk-0 bwd chunks reversed), bulk-1 with the
    layer-0 recurrence (h0 becomes available middle-out), and emissions
    (+exp +numerator partials) with the layer-1 recurrence.
  * CRF runs in probability space, split into a forward alpha chain
    (t=0..255) and a backward beta chain (t=511..256) that execute
    concurrently, halving the sequential CRF depth. State ping-pongs
    between PSUM (matmul out) and SBUF (elementwise mult with expem),
    with one ln-renorm per chain. em' = em + b_out - log(L); the log(L)
    shift cancels between numerator and denominator.
"""

import os
import sys

import numpy as np

sys.path.insert(0, "/opt/trn_rl_repo")

import concourse.bass as bass
import concourse.tile as tile
from concourse import bacc, mybir
from concourse.bass_utils import run_bass_kernel_spmd

B, T, V, D, HD, L = 64, 512, 100000, 300, 256, 9
H = 128
NCORES = 8
BL = B // NCORES          # sequences per core
DPAD = 384                # D padded so DMA-transpose chunks are 128 wide
KCH = (128, 128, 128)     # K chunks of DPAD
CBAR = float(np.log(L))   # per-step CRF shift (cancels in num - denom)

f32 = mybir.dt.float32
bf16 = mybir.dt.bfloat16
i32 = mybir.dt.int32
ALU = mybir.AluOpType
ACTF = mybir.ActivationFunctionType


# ---------------------------------------------------------------------------
# device program
# ---------------------------------------------------------------------------

def build_program(Tsteps=T):
    NT = Tsteps * BL
    NCK = max(1, NT // 512)            # bulk matmul N chunks
    NCOLS = NT // NCK
    NTILES = NT // 128                 # gather tiles
    assert NT % 128 == 0 and NT % NCK == 0

    nc = bacc.Bacc("TRN2", target_bir_lowering=False, debug=False)

    def din(name, shape, dt):
        return nc.dram_tensor(name, shape, dt, kind="ExternalInput").ap()

    words = din("words", [128, NTILES], i32)
    emb = din("emb", [V, DPAD], bf16)
    ident = din("ident", [128, 128], bf16)
    # lhsT weights, gate-major free dim (slots o,i,f,g each 128 wide)
    wih0 = din("wih0", [2, 3, 128, 512], bf16)     # [dir][kchunk][K][4*128]
    wih1 = din("wih1", [2, 2, 128, 512], bf16)     # [dir][h0-dir kchunk][K][4*128]
    whh = din("whh", [2, 2, 128, 512], bf16)       # [layer][dir][K=H][4*128]
    biases = din("biases", [2, 2, 128, 4], f32)    # [layer][dir][hidden][gate]
    woutT = din("woutT", [2, 128, L], bf16)        # [h1-dir kchunk][K][L]
    bout = din("bout", [L, 1], f32)                # b_out - CBAR
    estart = din("estart", [L, 1], f32)            # exp(start_t)
    expE = din("expE", [L, L], f32)                # exp(trans)
    expET = din("expET", [L, L], f32)              # exp(trans).T
    expend = din("expend", [L, 1], f32)            # exp(end_t)
    oh = din("oh", [L, NT], bf16)                  # label one-hot, (t,b) order
    res = nc.dram_tensor("res", [1, 2], f32, kind="ExternalOutput").ap()

    with tile.TileContext(nc) as tc:
        _emit(tc, nc, Tsteps, NT, NCK, NCOLS, NTILES,
              words, emb, ident, wih0, wih1, whh, biases, woutT, bout,
              estart, expE, expET, expend, oh, res)
    nc.compile()
    return nc


def _emit(tc, nc, Tsteps, NT, NCK, NCOLS, NTILES,
          words, emb, ident, wih0, wih1, whh, biases, woutT, bout,
          estart, expE, expET, expend, oh, res):
    from contextlib import ExitStack

    TM = Tsteps // 2            # alpha covers t<=TM-1... (split point)
    RENORM_K = 128              # renorm once per chain at this chain-step

    ctx = ExitStack()
    with ctx:
        consts = ctx.enter_context(tc.tile_pool(name="consts", bufs=1))
        states = ctx.enter_context(tc.tile_pool(name="states", bufs=1))
        # PSUM budget (8 banks): bulkp 2 + gates 2 + transpose 2 + alpha/beta 2
        bulkp = ctx.enter_context(
            tc.tile_pool(name="bulkp", bufs=2, space="PSUM"))
        gatesp = ctx.enter_context(
            tc.tile_pool(name="gatesp", bufs=1, space="PSUM"))
        tpp = ctx.enter_context(
            tc.tile_pool(name="tpp", bufs=2, space="PSUM"))
        crfp = ctx.enter_context(
            tc.tile_pool(name="crfp", bufs=1, space="PSUM"))
        wpool = ctx.enter_context(tc.tile_pool(name="wpool", bufs=4))
        scrp = ctx.enter_context(tc.tile_pool(name="scrp", bufs=2))
        crfs = ctx.enter_context(tc.tile_pool(name="crfs", bufs=4))
        xTp = ctx.enter_context(tc.tile_pool(name="xT", bufs=1))

        # ---- persistent SBUF tiles ----
        ident_sb = consts.tile([128, 128], bf16, tag="ident")
        nc.sync.dma_start(ident_sb[:], ident[:])
        whh_sb = {}
        for l in range(2):
            for d in range(2):
                t_ = consts.tile([128, 512], bf16, name=f"whh{l}{d}")
                nc.sync.dma_start(t_[:], whh[l, d])
                whh_sb[l, d] = t_
        bias_sb = {}
        for l in range(2):
            for d in range(2):
                t_ = consts.tile([128, 4], f32, name=f"bias{l}{d}")
                nc.sync.dma_start(t_[:], biases[l, d])
                bias_sb[l, d] = t_
        wih0_sb = {}
        for d in range(2):
            for c in range(3):
                t_ = consts.tile([128, 512], bf16, name=f"wih0_{d}{c}")
                nc.sync.dma_start(t_[:], wih0[d, c])
                wih0_sb[d, c] = t_
        wih1_sb = {}
        for d in range(2):
            for k in range(2):
                t_ = consts.tile([128, 512], bf16, name=f"wih1_{d}{k}")
                nc.sync.dma_start(t_[:], wih1[d, k])
                wih1_sb[d, k] = t_
        woutT_sb = [consts.tile([128, L], bf16, name=f"wo{k}") for k in range(2)]
        for k in range(2):
            nc.sync.dma_start(woutT_sb[k][:], woutT[k])
        bout_sb = consts.tile([L, 1], f32, name="bout_sb")
        estart_sb = consts.tile([L, 1], f32, name="estart_sb")
        expE_sb = consts.tile([L, L], f32, name="expE_sb")
        expET_sb = consts.tile([L, L], f32, name="expET_sb")
        expend_sb = consts.tile([L, 1], f32, name="expend_sb")
        ones9 = consts.tile([L, L], f32, name="ones9")
        nc.sync.dma_start(bout_sb[:], bout[:])
        nc.sync.dma_start(estart_sb[:], estart[:])
        nc.sync.dma_start(expE_sb[:], expE[:])
        nc.sync.dma_start(expET_sb[:], expET[:])
        nc.sync.dma_start(expend_sb[:], expend[:])
        nc.vector.memset(ones9[:], 1.0)

        h_hist = {}
        for l in range(2):
            for d in range(2):
                h_hist[l, d] = states.tile([128, NT], bf16, name=f"h{l}{d}")
        # per-dir persistent cell tile, cols [o i f g C] (x BL each); the
        # C slot makes the fused (v2|z) DVE op's in1 = [g C] contiguous.
        ycell = [states.tile([128, 5 * BL], f32, name=f"yc{d}") for d in range(2)]
        tcl_st = [states.tile([128, BL], f32, name=f"tcl{d}") for d in range(2)]

        # xp for the two directions of the current layer (reused across layers)
        xp_sb = [states.tile([128, 4 * NT], bf16, name=f"xp{d}") for d in range(2)]

        # CRF persistent tiles
        expem = states.tile([L, NT], f32, name="expem")
        oh_sb = states.tile([L, NT], bf16, name="oh_sb")
        nc.sync.dma_start(oh_sb[:], oh[:])
        P_a = states.tile([L, BL], f32, name="P_a")       # alpha (SBUF leg)
        u_b = states.tile([L, BL], f32, name="u_b")       # beta (SBUF leg)
        lnacc = states.tile([1, BL], f32, name="lnacc")
        num9c = states.tile([L, NCK], f32, name="num9c")
        num9 = states.tile([L, 1], f32, name="num9")
        nc.vector.memset(lnacc[:], 0.0)

        def bulk_chunk(layer, d, nck, srcs):
            """xp[d][:, chunk nck] = srcs-matmul + bias, gate slots (o,i,f,g).

            Matmuls and bias adds are split into sub-chunks so no single
            instruction blocks a latency-critical recurrence op for long.
            """
            nsl = slice(nck * NCOLS, (nck + 1) * NCOLS)
            xv = xp_sb[d][:].rearrange("p (t g b) -> p t g b", g=4, b=BL)
            tpc = NCOLS // BL
            NSUB = 4
            sub = NCOLS // NSUB
            tps = tpc // NSUB
            for slot in range(4):
                pt = bulkp.tile([128, NCOLS], f32, tag="pt", name="pt")
                for s in range(NSUB):
                    for ki, (src, wt, kk) in enumerate(srcs):
                        nc.tensor.matmul(
                            pt[:, s * sub:(s + 1) * sub],
                            lhsT=wt[:kk, slot * 128:(slot + 1) * 128],
                            rhs=src[:kk, nck * NCOLS + s * sub:
                                    nck * NCOLS + (s + 1) * sub],
                            start=(ki == 0), stop=(ki == len(srcs) - 1),
                        )
                    nc.vector.tensor_scalar(
                        out=xv[:, nck * tpc + s * tps:nck * tpc + (s + 1) * tps,
                               slot, :],
                        in0=pt[:, s * sub:(s + 1) * sub].rearrange(
                            "p (t b) -> p t b", b=BL),
                        scalar1=bias_sb[layer, d][:, slot:slot + 1],
                        scalar2=None, op0=ALU.add,
                    )

        def recur_phase(layer):
            # Two independent dir-chains; each cell is PE -> ACT -> DVE(w)
            # -> DVE(C) -> ACT -> DVE(h). Wall time = T x chain path; the
            # two chains overlap on the engines.
            for d in range(2):
                nc.vector.memset(ycell[d][:, 4 * BL:5 * BL], 0.0)
            for t in range(Tsteps):
                taus = (t, Tsteps - 1 - t)
                first = (t == 0)
                for d in range(2):
                    tau = taus[d]
                    y = ycell[d]
                    gp = gatesp.tile([128, 4 * BL], f32, tag=f"gp{d}",
                                     name=f"gp{d}")
                    nc.tensor.matmul(gp[:], lhsT=ident_sb[:],
                                     rhs=xp_sb[d][:, tau * 4 * BL:(tau + 1) * 4 * BL],
                                     start=True, stop=first)
                    if not first:
                        prev = tau - 1 if d == 0 else tau + 1
                        hh = h_hist[layer, d]
                        whh_t = whh_sb[layer, d]
                        for slot in range(4):
                            nc.tensor.matmul(
                                gp[:, slot * BL:(slot + 1) * BL],
                                lhsT=whh_t[:, slot * 128:(slot + 1) * 128],
                                rhs=hh[:, prev * BL:(prev + 1) * BL],
                                start=False, stop=(slot == 3))
                    # y[0:4BL] = tanh(gates/2), slots (o,i,f,g)
                    nc.scalar.activation(y[:, 0:4 * BL], gp[:], ACTF.Tanh,
                                         scale=0.5)
                    # w = [(yi+1)*yg | (yf+1)*C_old] = [v2 | z]
                    w = wpool.tile([128, 2 * BL], f32, tag=f"w{d}", name=f"w{d}")
                    nc.vector.scalar_tensor_tensor(
                        w[:], in0=y[:, BL:3 * BL], scalar=1.0,
                        in1=y[:, 3 * BL:5 * BL], op0=ALU.add, op1=ALU.mult)
                    # C = 0.5*z + v2
                    nc.vector.scalar_tensor_tensor(
                        y[:, 4 * BL:5 * BL], in0=w[:, BL:2 * BL], scalar=0.5,
                        in1=w[:, 0:BL], op0=ALU.mult, op1=ALU.add)
                    nc.scalar.activation(tcl_st[d][:], y[:, 4 * BL:5 * BL],
                                         ACTF.Tanh, scale=0.5)
                    nc.vector.scalar_tensor_tensor(
                        h_hist[layer, d][:, tau * BL:(tau + 1) * BL],
                        in0=y[:, 0:BL], scalar=1.0, in1=tcl_st[d][:],
                        op0=ALU.add, op1=ALU.mult)

        # =================================================================
        # Phase 1: embedding gather + transpose (emitted from both sequence
        # ends so both bulk-0 dir-chunk streams start early)
        # =================================================================
        x_T = [xTp.tile([k, NT], bf16, name=f"xT{c}") for c, k in enumerate(KCH)]
        idx_all = consts.tile([128, NTILES], i32, name="idx_all")
        nc.sync.dma_start(idx_all[:], words[:])
        with tc.tile_pool(name="gath", bufs=4) as gp_:
            order = []
            lo, hi = 0, NTILES - 1
            while lo <= hi:
                order.append(lo)
                if hi != lo:
                    order.append(hi)
                lo, hi = lo + 1, hi - 1
            for i in order:
                g = gp_.tile([128, DPAD], bf16, tag="g", name="g")
                nc.gpsimd.indirect_dma_start(
                    out=g[:], out_offset=None, in_=emb[:],
                    in_offset=bass.IndirectOffsetOnAxis(ap=idx_all[:, i:i + 1],
                                                        axis=0),
                )
                # transpose on the (idle) PE + DVE copy-back: keeps the
                # HWDGE free so it never stalls the frozen PE stream.
                for c, k in enumerate(KCH):
                    tp = tpp.tile([128, 128], bf16, tag="tp", name="tp")
                    nc.tensor.transpose(tp[:], g[:, c * 128:(c + 1) * 128],
                                        ident_sb[:])
                    nc.vector.tensor_copy(out=x_T[c][:, i * 128:(i + 1) * 128],
                                          in_=tp[:])

        # bulk-0: fwd chunks ascending, bwd chunks descending, interleaved.
        # Deprioritized so the scheduler runs recurrence ops first in any
        # engine-idle gap (bulk fills the slack; data deps still hold).
        LOW = -10_000_000
        srcs0 = lambda d: [(x_T[c], wih0_sb[d, c], KCH[c]) for c in range(3)]
        with tc.high_priority(offset=LOW):
            for j in range(NCK):
                bulk_chunk(0, 0, j, srcs0(0))
                bulk_chunk(0, 1, NCK - 1 - j, srcs0(1))

        recur_phase(0)

        # bulk-1 middle-out: h0 regions complete middle-out during recur 0,
        # so these run concurrently with the tail of the layer-0 recurrence.
        srcs1 = lambda d: [(h_hist[0, k], wih1_sb[d, k], 128) for k in range(2)]
        mid_order = []
        lo, hi = NCK // 2 - 1, NCK // 2
        while lo >= 0:
            mid_order.extend([hi, lo])
            lo, hi = lo - 1, hi + 1
        with tc.high_priority(offset=LOW):
            for j in mid_order:
                for d in range(2):
                    bulk_chunk(1, d, j, srcs1(d))

        recur_phase(1)

        # =================================================================
        # Emissions (middle-out, overlap recur 1): per chunk
        #   pt = w_out @ h1 (PSUM); expem = exp(pt + bout') (ACT);
        #   num partial = sum(pt * onehot) via accum_out (DVE)
        # =================================================================
        with tc.high_priority(offset=LOW):
            for nck in mid_order:
                nsl = slice(nck * NCOLS, (nck + 1) * NCOLS)
                pt = bulkp.tile([L, NCOLS], f32, tag="pt", name="pt")
                for k in range(2):
                    nc.tensor.matmul(pt[:], lhsT=woutT_sb[k][:],
                                     rhs=h_hist[1, k][:, nsl],
                                     start=(k == 0), stop=(k == 1))
                nc.scalar.activation(expem[:, nsl], pt[:], ACTF.Exp,
                                     bias=bout_sb[:, 0:1])
                scr = scrp.tile([L, NCOLS], f32, tag="scr", name="scr")
                nc.vector.scalar_tensor_tensor(
                    scr[:], in0=pt[:], scalar=0.0, in1=oh_sb[:, nsl],
                    op0=ALU.add, op1=ALU.mult,
                    accum_out=num9c[:, nck:nck + 1])

        # =================================================================
        # CRF: two-ended scan in probability space.
        #   alpha: P_{t} = (E^T P_{t-1}) o expem_t   for t = 1..TM-1
        #   beta:  Q_{t} = E (expem_{t+1} o Q_{t+1}) for t = T-2..TM-1
        #   Z_b = sum_i alpha_{TM-1}[i] * beta_{TM-1}[i]
        # State ping-pongs PSUM (matmul) <-> SBUF (mult); one renorm each.
        # =================================================================
        def renorm(vec_sb, b_lnacc_col):
            srow = bulkp.tile([L, BL], f32, tag="pt", name="srow")
            nc.tensor.matmul(srow[:], lhsT=ones9[:], rhs=vec_sb[:],
                             start=True, stop=True)
            lns = crfs.tile([1, BL], f32, tag="lns", name="lns")
            nc.scalar.activation(lns[:], srow[0:1, :], ACTF.Ln)
            nc.vector.tensor_tensor(out=lnacc[:], in0=lnacc[:], in1=lns[:],
                                    op=ALU.add)
            rec = crfs.tile([L, BL], f32, tag="rec", name="rec")
            nc.vector.reciprocal(rec[:], srow[:])
            nc.vector.tensor_tensor(out=vec_sb[:], in0=vec_sb[:], in1=rec[:],
                                    op=ALU.mult)

        # alpha init: P_0 = exp(start) o expem_0
        nc.vector.tensor_scalar(out=P_a[:], in0=expem[:, 0:BL],
                                scalar1=estart_sb[:, 0:1], scalar2=None,
                                op0=ALU.mult)
        # beta init: u = expem_{T-1} o expend  (u for producing Q_{T-2})
        nc.vector.tensor_scalar(out=u_b[:], in0=expem[:, (Tsteps - 1) * BL:],
                                scalar1=expend_sb[:, 0:1], scalar2=None,
                                op0=ALU.mult)
        # chains: alpha consumes expem_1..expem_{TM-1};
        # beta consumes expem_{T-2}..expem_{TM} then combines at TM-1.
        n_a = TM - 1                 # alpha steps (t = 1..TM-1)
        n_b = Tsteps - TM            # beta matmul steps producing Q_{TM-1}
        for k in range(1, max(n_a, n_b) + 1):
            if k <= n_a:
                sp = crfp.tile([L, BL], f32, tag="sp", name="sp")
                nc.tensor.matmul(sp[:], lhsT=expE_sb[:], rhs=P_a[:],
                                 start=True, stop=True)
                nc.vector.tensor_tensor(
                    out=P_a[:], in0=sp[:],
                    in1=expem[:, k * BL:(k + 1) * BL], op=ALU.mult)
                if k == RENORM_K:
                    renorm(P_a, 0)
            if k <= n_b:
                sq = crfp.tile([L, BL], f32, tag="sq", name="sq")
                nc.tensor.matmul(sq[:], lhsT=expET_sb[:], rhs=u_b[:],
                                 start=True, stop=True)
                t_next = Tsteps - 1 - k      # Q_{t_next} now in sq
                if k < n_b:
                    nc.vector.tensor_tensor(
                        out=u_b[:], in0=sq[:],
                        in1=expem[:, t_next * BL:(t_next + 1) * BL],
                        op=ALU.mult)
                    if k == RENORM_K:
                        renorm(u_b, 0)
                else:
                    # final: u_b <- Q_{TM-1} (plain copy out of PSUM)
                    nc.vector.tensor_copy(out=u_b[:], in_=sq[:])

        # combine: Z_b = sum_i P_a[i,b] * u_b[i,b]; denom = ln Z + lnacc
        ab = crfs.tile([L, BL], f32, tag="ab", name="ab")
        nc.vector.tensor_tensor(out=ab[:], in0=P_a[:], in1=u_b[:], op=ALU.mult)
        zrow = bulkp.tile([L, BL], f32, tag="pt", name="zrow")
        nc.tensor.matmul(zrow[0:1, :], lhsT=ones9[:, 0:1], rhs=ab[:],
                         start=True, stop=True)
        lnz = crfs.tile([1, BL], f32, tag="lns", name="lnz")
        nc.scalar.activation(lnz[:], zrow[0:1, :], ACTF.Ln)
        nc.vector.tensor_tensor(out=lnz[:], in0=lnz[:], in1=lnacc[:],
                                op=ALU.add)
        dsc = crfs.tile([1, 1], f32, tag="dsc", name="dsc")
        nc.vector.tensor_reduce(dsc[:], lnz[:], axis=mybir.AxisListType.X,
                                op=ALU.add)
        nc.vector.tensor_reduce(num9[:], num9c[:], axis=mybir.AxisListType.X,
                                op=ALU.add)
        npsum = bulkp.tile([L, BL], f32, tag="pt", name="npsum")
        nc.tensor.matmul(npsum[0:1, 0:1], lhsT=ones9[:, 0:1], rhs=num9[:, 0:1],
                         start=True, stop=True)
        out_sb = crfs.tile([1, 2], f32, tag="out_sb", name="out_sb")
        nc.vector.tensor_scalar(out=out_sb[:, 0:1], in0=npsum[0:1, 0:1],
                                scalar1=0.0, scalar2=None, op0=ALU.add)
        nc.vector.tensor_scalar(out=out_sb[:, 1:2], in0=dsc[:],
                                scalar1=0.0, scalar2=None, op0=ALU.add)
        nc.sync.dma_start(res[:], out_sb[:])


# ---------------------------------------------------------------------------
# host side
# ---------------------------------------------------------------------------

def _prescale(w_ih, w_hh, b_ih, b_hh, h_in_doubled):
    """Gate-slot layout is torch order (i,f,g,o). Returns fp32 arrays."""
    sg = np.ones((4, 1), np.float32)
    sg[2] = 2.0                       # g gate rows x2 (tanh scale 0.5 trick)
    srows = np.repeat(sg, H, axis=0)  # [512, 1]
    wih = w_ih.astype(np.float32) * srows
    whh_ = w_hh.astype(np.float32) * srows * 0.5
    b = (b_ih + b_hh).astype(np.float32) * srows[:, 0]
    if h_in_doubled:
        wih = wih * 0.5
    return wih, whh_, b


GATE_ORDER = (3, 0, 1, 2)   # device slot s holds torch gate GATE_ORDER[s]: o,i,f,g


def _lhsT_gate_major(w, kchunks):
    """w: [4H, K] fp32 -> [nchunk, 128, 512] bf16 lhsT (zero-padded K)."""
    outs = []
    off = 0
    for kk in kchunks:
        blk = np.zeros((128, 512), np.float32)
        take = min(kk, w.shape[1] - off)
        for slot, g in enumerate(GATE_ORDER):
            blk[:take, slot * 128:(slot + 1) * 128] = \
                w[g * H:(g + 1) * H, off:off + take].T
        outs.append(blk)
        off += kk
    return np.stack(outs).astype(np.dtype("bfloat16"))


_PROG_CACHE = {}


def _get_program(Tsteps):
    if Tsteps not in _PROG_CACHE:
        _PROG_CACHE[Tsteps] = build_program(Tsteps)
    return _PROG_CACHE[Tsteps]


def prepare_inputs(inputs, Tsteps=T):
    """Build the per-core input maps + the host numerator constants."""
    bfl = np.dtype("bfloat16")
    words = np.asarray(inputs["word_batch"]).astype(np.int64)
    labels = np.asarray(inputs["label_batch"]).astype(np.int64)
    emb = np.asarray(inputs["emb"], np.float32)
    words = words[:, :Tsteps]
    labels = labels[:, :Tsteps]

    embp = np.zeros((V, DPAD), np.float32)
    embp[:, :D] = emb
    embp = embp.astype(bfl)

    ident = np.eye(128, dtype=np.float32).astype(bfl)

    wih0_l, whh_l, wih1_l, bias_l = [], [], [], []
    for layer, (wihk, whhk, bihk, bhhk) in enumerate(
            [("w_ih_l0", "w_hh_l0", "b_ih_l0", "b_hh_l0"),
             ("w_ih_l1", "w_hh_l1", "b_ih_l1", "b_hh_l1")]):
        for d in range(2):
            wih, whh_, b = _prescale(
                np.asarray(inputs[wihk])[d], np.asarray(inputs[whhk])[d],
                np.asarray(inputs[bihk])[d], np.asarray(inputs[bhhk])[d],
                h_in_doubled=(layer == 1))
            if layer == 0:
                wihp = np.zeros((512, DPAD), np.float32)
                wihp[:, :D] = wih
                wih0_l.append(_lhsT_gate_major(wihp, KCH))
            else:
                wih1_l.append(_lhsT_gate_major(wih, (128, 128)))
            whh_l.append(_lhsT_gate_major(whh_, (128,)))
            bias_l.append(b.reshape(4, H)[list(GATE_ORDER)].T)  # [128, 4]
    wih0 = np.stack(wih0_l)                       # [2, 3, 128, 512]
    wih1 = np.stack(wih1_l)                       # [2, 2, 128, 512]
    whh = np.stack(whh_l).reshape(2, 2, 1, 128, 512)[:, :, 0]
    biases = np.stack(bias_l).reshape(2, 2, 128, 4).astype(np.float32)

    w_out = np.asarray(inputs["w_out"], np.float32) * 0.5   # [L, 2H]
    woutT = np.stack([w_out[:, :H].T, w_out[:, H:].T]).astype(bfl)  # [2,128,L]
    boutp = (np.asarray(inputs["b_out"], np.float32) - CBAR).reshape(L, 1)
    estart = np.exp(np.asarray(inputs["start_t"], np.float32)).reshape(L, 1)
    expEm = np.exp(np.asarray(inputs["trans"], np.float32))
    expend = np.exp(np.asarray(inputs["end_t"], np.float32)).reshape(L, 1)

    NT = Tsteps * BL
    in_maps = []
    num_consts = []
    start_t = np.asarray(inputs["start_t"], np.float32)
    end_t = np.asarray(inputs["end_t"], np.float32)
    trans = np.asarray(inputs["trans"], np.float32)
    bshift = np.asarray(inputs["b_out"], np.float32) - CBAR   # device em offset
    for c in range(NCORES):
        bs = slice(c * BL, (c + 1) * BL)
        wc = words[bs]                        # [BL, Tsteps]
        lc = labels[bs]
        toks = wc.T.reshape(-1).astype(np.int32)          # (t, b) order
        ohc = (lc.T.reshape(1, -1) == np.arange(L).reshape(L, 1))
        in_maps.append({
            "words": np.ascontiguousarray(toks.reshape(NT // 128, 128).T),
            "emb": embp, "ident": ident,
            "wih0": wih0, "wih1": wih1, "whh": whh, "biases": biases,
            "woutT": woutT, "bout": boutp, "estart": estart,
            "expE": expEm, "expET": np.ascontiguousarray(expEm.T),
            "expend": expend,
            "oh": ohc.astype(bfl),
        })
        num_consts.append(
            float(start_t[lc[:, 0]].sum())
            + float(trans[lc[:, :-1], lc[:, 1:]].sum())
            + float(end_t[lc[:, -1]].sum())
            + float(bshift[lc].sum()))        # device num omits the em bias
    return in_maps, num_consts


def kernel(**inputs):
    in_maps, num_consts = prepare_inputs(inputs, T)
    nc = _get_program(T)
    out = run_bass_kernel_spmd(nc, in_maps, list(range(NCORES)))
    llh = 0.0
    for c in range(NCORES):
        r = out.results[c]["res"].reshape(2).astype(np.float64)
        llh += num_consts[c] + r[0] - r[1]
    return np.float32(-llh)


if __name__ == "__main__":
    np.random.seed(0)
    print("building program (small T) ...")
    build_program(16)
    print("ok")


# revision 23
# speedup vs baseline: 1.0162x; 1.0067x over previous
"""BiLSTM-CRF forward NLL on 8 Trainium2 NeuronCores.

Sharding: pure data-parallel over batch (8 sequences per core), params
replicated. Per core: embedding gather -> bulk input matmuls -> 2-layer
BiLSTM recurrence (fwd/bwd chains interleaved per layer) -> emissions ->
CRF scan -> partial (num, denom) pair. Host sums partials with the
label-dependent numerator constant.

Key restructurings (validated against the reference):
  * LSTM cell uses a single tanh activation per step over all 4 gates:
    sigmoid(x) = (tanh(x/2)+1)/2, with the tanh(0.5*...) instruction scale
    folded into host-prescaled weights (g-gate rows x2). The cell tracks
    C = 2c and H = 2h; every consumer of h (recurrent weights, layer-1
    input weights, output projection) is pre-halved on the host.
  * Gate slots are laid out (o,i,f,g) with a per-dir persistent cell tile
    [o i f g C] so one fused DVE op computes v2=(yi+1)*yg and z=(yf+1)*C.
  * All tile pools coexist (PSUM banks budgeted to 8) so the readiness-
    driven scheduler overlaps: gather with bulk-0 (gather emitted from
    both sequence ends, bulk-0 bwd chunks reversed), bulk-1 with the
    layer-0 recurrence (h0 becomes available middle-out), and emissions
    (+exp +numerator partials) with the layer-1 recurrence.
  * CRF runs in probability space, split into a forward alpha chain
    (t=0..255) and a backward beta chain (t=511..256) that execute
    concurrently, halving the sequential CRF depth. State ping-pongs
    between PSUM (matmul out) and SBUF (elementwise mult with expem),
    with one ln-renorm per chain. em' = em + b_out - log(L); the log(L)
    shift cancels between numerator and denominator.
"""

import os
import sys

import numpy as np

sys.path.insert(0, "/opt/trn_rl_repo")

import concourse.bass as bass
import concourse.tile as tile
from concourse import bacc, mybir
from concourse.bass_utils import run_bass_kernel_spmd

B, T, V, D, HD, L = 64, 512, 100000, 300, 256, 9
H = 128
NCORES = 8
BL = B // NCORES          # sequences per core
DPAD = 384                # D padded so DMA-transpose chunks are 128 wide
KCH = (128, 128, 128)     # K chunks of DPAD
CBAR = float(np.log(L))   # per-step CRF shift (cancels in num - denom)

f32 = mybir.dt.float32
bf16 = mybir.dt.bfloat16
i32 = mybir.dt.int32
ALU = mybir.AluOpType
ACTF = mybir.ActivationFunctionType


# ---------------------------------------------------------------------------
# device program
# ---------------------------------------------------------------------------

def build_program(Tsteps=T):
    NT = Tsteps * BL
    NCK = max(1, NT // 512)            # bulk matmul N chunks
    NCOLS = NT // NCK
    NTILES = NT // 128                 # gather tiles
    assert NT % 128 == 0 and NT % NCK == 0

    nc = bacc.Bacc("TRN2", target_bir_lowering=False, debug=False)

    def din(name, shape, dt):
        return nc.dram_tensor(name, shape, dt, kind="ExternalInput").ap()

    words = din("words", [128, NTILES], i32)
    emb = din("emb", [V, DPAD], bf16)
    ident = din("ident", [128, 128], bf16)
    # lhsT weights, gate-major free dim (slots o,i,f,g each 128 wide)
    wih0 = din("wih0", [2, 3, 128, 512], bf16)     # [dir][kchunk][K][4*128]
    wih1 = din("wih1", [2, 2, 128, 512], bf16)     # [dir][h0-dir kchunk][K][4*128]
    whh = din("whh", [2, 2, 128, 512], bf16)       # [layer][dir][K=H][4*128]
    biases = din("biases", [2, 2, 128, 4], f32)    # [layer][dir][hidden][gate]
    woutT = din("woutT", [2, 128, L], bf16)        # [h1-dir kchunk][K][L]
    bout = din("bout", [L, 1], f32)                # b_out - CBAR
    estart = din("estart", [L, 1], f32)            # exp(start_t)
    expE = din("expE", [L, L], f32)                # exp(trans)
    expET = din("expET", [L, L], f32)              # exp(trans).T
    expend = din("expend", [L, 1], f32)            # exp(end_t)
    oh = din("oh", [L, NT], bf16)                  # label one-hot, (t,b) order
    res = nc.dram_tensor("res", [1, 2], f32, kind="ExternalOutput").ap()

    with tile.TileContext(nc) as tc:
        _emit(tc, nc, Tsteps, NT, NCK, NCOLS, NTILES,
              words, emb, ident, wih0, wih1, whh, biases, woutT, bout,
              estart, expE, expET, expend, oh, res)
    nc.compile()
    return nc


def _emit(tc, nc, Tsteps, NT, NCK, NCOLS, NTILES,
          words, emb, ident, wih0, wih1, whh, biases, woutT, bout,
          estart, expE, expET, expend, oh, res):
    from contextlib import ExitStack

    TM = Tsteps // 2            # alpha covers t<=TM-1... (split point)
    RENORM_K = 128              # renorm once per chain at this chain-step

    ctx = ExitStack()
    with ctx:
        consts = ctx.enter_context(tc.tile_pool(name="consts", bufs=1))
        states = ctx.enter_context(tc.tile_pool(name="states", bufs=1))
        # PSUM budget (8 banks): bulkp 2 + gates 2 + transpose 2 + alpha/beta 2
        bulkp = ctx.enter_context(
            tc.tile_pool(name="bulkp", bufs=2, space="PSUM"))
        gatesp = ctx.enter_context(
            tc.tile_pool(name="gatesp", bufs=1, space="PSUM"))
        tpp = ctx.enter_context(
            tc.tile_pool(name="tpp", bufs=2, space="PSUM"))
        crfp = ctx.enter_context(
            tc.tile_pool(name="crfp", bufs=1, space="PSUM"))
        wpool = ctx.enter_context(tc.tile_pool(name="wpool", bufs=4))
        scrp = ctx.enter_context(tc.tile_pool(name="scrp", bufs=2))
        crfs = ctx.enter_context(tc.tile_pool(name="crfs", bufs=4))
        xTp = ctx.enter_context(tc.tile_pool(name="xT", bufs=1))

        # ---- persistent SBUF tiles ----
        ident_sb = consts.tile([128, 128], bf16, tag="ident")
        nc.sync.dma_start(ident_sb[:], ident[:])
        whh_sb = {}
        for l in range(2):
            for d in range(2):
                t_ = consts.tile([128, 512], bf16, name=f"whh{l}{d}")
                nc.sync.dma_start(t_[:], whh[l, d])
                whh_sb[l, d] = t_
        bias_sb = {}
        for l in range(2):
            for d in range(2):
                t_ = consts.tile([128, 4], f32, name=f"bias{l}{d}")
                nc.sync.dma_start(t_[:], biases[l, d])
                bias_sb[l, d] = t_
        wih0_sb = {}
        for d in range(2):
            for c in range(3):
                t_ = consts.tile([128, 512], bf16, name=f"wih0_{d}{c}")
                nc.sync.dma_start(t_[:], wih0[d, c])
                wih0_sb[d, c] = t_
        wih1_sb = {}
        for d in range(2):
            for k in range(2):
                t_ = consts.tile([128, 512], bf16, name=f"wih1_{d}{k}")
                nc.sync.dma_start(t_[:], wih1[d, k])
                wih1_sb[d, k] = t_
        woutT_sb = [consts.tile([128, L], bf16, name=f"wo{k}") for k in range(2)]
        for k in range(2):
            nc.sync.dma_start(woutT_sb[k][:], woutT[k])
        bout_sb = consts.tile([L, 1], f32, name="bout_sb")
        estart_sb = consts.tile([L, 1], f32, name="estart_sb")
        expE_sb = consts.tile([L, L], f32, name="expE_sb")
        expET_sb = consts.tile([L, L], f32, name="expET_sb")
        expend_sb = consts.tile([L, 1], f32, name="expend_sb")
        ones9 = consts.tile([L, L], f32, name="ones9")
        nc.sync.dma_start(bout_sb[:], bout[:])
        nc.sync.dma_start(estart_sb[:], estart[:])
        nc.sync.dma_start(expE_sb[:], expE[:])
        nc.sync.dma_start(expET_sb[:], expET[:])
        nc.sync.dma_start(expend_sb[:], expend[:])
        nc.vector.memset(ones9[:], 1.0)

        h_hist = {}
        for l in range(2):
            for d in range(2):
                h_hist[l, d] = states.tile([128, NT], bf16, name=f"h{l}{d}")
        # per-dir persistent cell tile, cols [o i f g C] (x BL each); the
        # C slot makes the fused (v2|z) DVE op's in1 = [g C] contiguous.
        ycell = [states.tile([128, 5 * BL], f32, name=f"yc{d}") for d in range(2)]
        tcl_st = [states.tile([128, BL], f32, name=f"tcl{d}") for d in range(2)]

        # xp for the two directions of the current layer (reused across layers)
        xp_sb = [states.tile([128, 4 * NT], bf16, name=f"xp{d}") for d in range(2)]

        # CRF persistent tiles
        expem = states.tile([L, NT], f32, name="expem")
        oh_sb = states.tile([L, NT], bf16, name="oh_sb")
        nc.sync.dma_start(oh_sb[:], oh[:])
        P_a = states.tile([L, BL], f32, name="P_a")       # alpha (SBUF leg)
        u_b = states.tile([L, BL], f32, name="u_b")       # beta (SBUF leg)
        lnacc = states.tile([1, BL], f32, name="lnacc")
        num9c = states.tile([L, NCK], f32, name="num9c")
        num9 = states.tile([L, 1], f32, name="num9")
        nc.vector.memset(lnacc[:], 0.0)

        def bulk_chunk(layer, d, nck, srcs):
            """xp[d][:, chunk nck] = srcs-matmul + bias, gate slots (o,i,f,g)."""
            nsl = slice(nck * NCOLS, (nck + 1) * NCOLS)
            xv = xp_sb[d][:].rearrange("p (t g b) -> p t g b", g=4, b=BL)
            tpc = NCOLS // BL
            for slot in range(4):
                pt = bulkp.tile([128, NCOLS], f32, tag="pt", name="pt")
                for ki, (src, wt, kk) in enumerate(srcs):
                    nc.tensor.matmul(
                        pt[:],
                        lhsT=wt[:kk, slot * 128:(slot + 1) * 128],
                        rhs=src[:kk, nsl],
                        start=(ki == 0), stop=(ki == len(srcs) - 1),
                    )
                nc.vector.tensor_scalar(
                    out=xv[:, nck * tpc:(nck + 1) * tpc, slot, :],
                    in0=pt[:].rearrange("p (t b) -> p t b", b=BL),
                    scalar1=bias_sb[layer, d][:, slot:slot + 1],
                    scalar2=None, op0=ALU.add,
                )

        def recur_phase(layer):
            # Two independent dir-chains; each cell is PE -> ACT -> DVE(w)
            # -> DVE(C) -> ACT -> DVE(h). Wall time = T x chain path; the
            # two chains overlap on the engines.
            for d in range(2):
                nc.vector.memset(ycell[d][:, 4 * BL:5 * BL], 0.0)
            for t in range(Tsteps):
                taus = (t, Tsteps - 1 - t)
                first = (t == 0)
                for d in range(2):
                    tau = taus[d]
                    y = ycell[d]
                    gp = gatesp.tile([128, 4 * BL], f32, tag=f"gp{d}",
                                     name=f"gp{d}")
                    nc.tensor.matmul(gp[:], lhsT=ident_sb[:],
                                     rhs=xp_sb[d][:, tau * 4 * BL:(tau + 1) * 4 * BL],
                                     start=True, stop=first)
                    if not first:
                        prev = tau - 1 if d == 0 else tau + 1
                        hh = h_hist[layer, d]
                        whh_t = whh_sb[layer, d]
                        for slot in range(4):
                            nc.tensor.matmul(
                                gp[:, slot * BL:(slot + 1) * BL],
                                lhsT=whh_t[:, slot * 128:(slot + 1) * 128],
                                rhs=hh[:, prev * BL:(prev + 1) * BL],
                                start=False, stop=(slot == 3))
                    # y[0:4BL] = tanh(gates/2), slots (o,i,f,g)
                    nc.scalar.activation(y[:, 0:4 * BL], gp[:], ACTF.Tanh,
                                         scale=0.5)
                    # w = [(yi+1)*yg | (yf+1)*C_old] = [v2 | z]
                    w = wpool.tile([128, 2 * BL], f32, tag=f"w{d}", name=f"w{d}")
                    nc.vector.scalar_tensor_tensor(
                        w[:], in0=y[:, BL:3 * BL], scalar=1.0,
                        in1=y[:, 3 * BL:5 * BL], op0=ALU.add, op1=ALU.mult)
                    # C = 0.5*z + v2
                    nc.vector.scalar_tensor_tensor(
                        y[:, 4 * BL:5 * BL], in0=w[:, BL:2 * BL], scalar=0.5,
                        in1=w[:, 0:BL], op0=ALU.mult, op1=ALU.add)
                    nc.scalar.activation(tcl_st[d][:], y[:, 4 * BL:5 * BL],
                                         ACTF.Tanh, scale=0.5)
                    nc.vector.scalar_tensor_tensor(
                        h_hist[layer, d][:, tau * BL:(tau + 1) * BL],
                        in0=y[:, 0:BL], scalar=1.0, in1=tcl_st[d][:],
                        op0=ALU.add, op1=ALU.mult)

        # =================================================================
        # Phase 1: embedding gather + transpose (emitted from both sequence
        # ends so both bulk-0 dir-chunk streams start early)
        # =================================================================
        x_T = [xTp.tile([k, NT], bf16, name=f"xT{c}") for c, k in enumerate(KCH)]
        idx_all = consts.tile([128, NTILES], i32, name="idx_all")
        nc.sync.dma_start(idx_all[:], words[:])
        with tc.tile_pool(name="gath", bufs=4) as gp_:
            order = []
            lo, hi = 0, NTILES - 1
            while lo <= hi:
                order.append(lo)
                if hi != lo:
                    order.append(hi)
                lo, hi = lo + 1, hi - 1
            for i in order:
                g = gp_.tile([128, DPAD], bf16, tag="g", name="g")
                nc.gpsimd.indirect_dma_start(
                    out=g[:], out_offset=None, in_=emb[:],
                    in_offset=bass.IndirectOffsetOnAxis(ap=idx_all[:, i:i + 1],
                                                        axis=0),
                )
                # transpose on the (idle) PE + DVE copy-back: keeps the
                # HWDGE free so it never stalls the frozen PE stream.
                for c, k in enumerate(KCH):
                    tp = tpp.tile([128, 128], bf16, tag="tp", name="tp")
                    nc.tensor.transpose(tp[:], g[:, c * 128:(c + 1) * 128],
                                        ident_sb[:])
                    nc.vector.tensor_copy(out=x_T[c][:, i * 128:(i + 1) * 128],
                                          in_=tp[:])

        # bulk-0: fwd chunks ascending, bwd chunks descending, interleaved.
        # Deprioritized so the scheduler runs recurrence ops first in any
        # engine-idle gap (bulk fills the slack; data deps still hold).
        LOW = -10_000_000
        srcs0 = lambda d: [(x_T[c], wih0_sb[d, c], KCH[c]) for c in range(3)]
        with tc.high_priority(offset=LOW):
            for j in range(NCK):
                bulk_chunk(0, 0, j, srcs0(0))
                bulk_chunk(0, 1, NCK - 1 - j, srcs0(1))

        recur_phase(0)

        # bulk-1 middle-out: h0 regions complete middle-out during recur 0,
        # so these run concurrently with the tail of the layer-0 recurrence.
        srcs1 = lambda d: [(h_hist[0, k], wih1_sb[d, k], 128) for k in range(2)]
        mid_order = []
        lo, hi = NCK // 2 - 1, NCK // 2
        while lo >= 0:
            mid_order.extend([hi, lo])
            lo, hi = lo - 1, hi + 1
        with tc.high_priority(offset=LOW):
            for j in mid_order:
                for d in range(2):
                    bulk_chunk(1, d, j, srcs1(d))

        recur_phase(1)

        # =================================================================
        # Emissions (middle-out, overlap recur 1): per chunk
        #   pt = w_out @ h1 (PSUM); expem = exp(pt + bout') (ACT);
        #   num partial = sum(pt * onehot) via accum_out (DVE)
        # =================================================================
        with tc.high_priority(offset=LOW):
            for nck in mid_order:
                nsl = slice(nck * NCOLS, (nck + 1) * NCOLS)
                pt = bulkp.tile([L, NCOLS], f32, tag="pt", name="pt")
                for k in range(2):
                    nc.tensor.matmul(pt[:], lhsT=woutT_sb[k][:],
                                     rhs=h_hist[1, k][:, nsl],
                                     start=(k == 0), stop=(k == 1))
                nc.scalar.activation(expem[:, nsl], pt[:], ACTF.Exp,
                                     bias=bout_sb[:, 0:1])
                scr = scrp.tile([L, NCOLS], f32, tag="scr", name="scr")
                nc.vector.scalar_tensor_tensor(
                    scr[:], in0=pt[:], scalar=0.0, in1=oh_sb[:, nsl],
                    op0=ALU.add, op1=ALU.mult,
                    accum_out=num9c[:, nck:nck + 1])

        # =================================================================
        # CRF: two-ended scan in probability space.
        #   alpha: P_{t} = (E^T P_{t-1}) o expem_t   for t = 1..TM-1
        #   beta:  Q_{t} = E (expem_{t+1} o Q_{t+1}) for t = T-2..TM-1
        #   Z_b = sum_i alpha_{TM-1}[i] * beta_{TM-1}[i]
        # State ping-pongs PSUM (matmul) <-> SBUF (mult); one renorm each.
        # =================================================================
        def renorm(vec_sb, b_lnacc_col):
            srow = bulkp.tile([L, BL], f32, tag="pt", name="srow")
            nc.tensor.matmul(srow[:], lhsT=ones9[:], rhs=vec_sb[:],
                             start=True, stop=True)
            lns = crfs.tile([1, BL], f32, tag="lns", name="lns")
            nc.scalar.activation(lns[:], srow[0:1, :], ACTF.Ln)
            nc.vector.tensor_tensor(out=lnacc[:], in0=lnacc[:], in1=lns[:],
                                    op=ALU.add)
            rec = crfs.tile([L, BL], f32, tag="rec", name="rec")
            nc.vector.reciprocal(rec[:], srow[:])
            nc.vector.tensor_tensor(out=vec_sb[:], in0=vec_sb[:], in1=rec[:],
                                    op=ALU.mult)

        # alpha init: P_0 = exp(start) o expem_0
        nc.vector.tensor_scalar(out=P_a[:], in0=expem[:, 0:BL],
                                scalar1=estart_sb[:, 0:1], scalar2=None,
                                op0=ALU.mult)
        # beta init: u = expem_{T-1} o expend  (u for producing Q_{T-2})
        nc.vector.tensor_scalar(out=u_b[:], in0=expem[:, (Tsteps - 1) * BL:],
                                scalar1=expend_sb[:, 0:1], scalar2=None,
                                op0=ALU.mult)
        # chains: alpha consumes expem_1..expem_{TM-1};
        # beta consumes expem_{T-2}..expem_{TM} then combines at TM-1.
        n_a = TM - 1                 # alpha steps (t = 1..TM-1)
        n_b = Tsteps - TM            # beta matmul steps producing Q_{TM-1}
        for k in range(1, max(n_a, n_b) + 1):
            if k <= n_a:
                sp = crfp.tile([L, BL], f32, tag="sp", name="sp")
                nc.tensor.matmul(sp[:], lhsT=expE_sb[:], rhs=P_a[:],
                                 start=True, stop=True)
                nc.vector.tensor_tensor(
                    out=P_a[:], in0=sp[:],
                    in1=expem[:, k * BL:(k + 1) * BL], op=ALU.mult)
                if k == RENORM_K:
                    renorm(P_a, 0)
            if k <= n_b:
                sq = crfp.tile([L, BL], f32, tag="sq", name="sq")
                nc.tensor.matmul(sq[:], lhsT=expET_sb[:], rhs=u_b[:],
                                 start=True, stop=True)
                t_next = Tsteps - 1 - k      # Q_{t_next} now in sq
                if k < n_b:
                    nc.vector.tensor_tensor(
                        out=u_b[:], in0=sq[:],
                        in1=expem[:, t_next * BL:(t_next + 1) * BL],
                        op=ALU.mult)
                    if k == RENORM_K:
                        renorm(u_b, 0)
                else:
                    # final: u_b <- Q_{TM-1} (plain copy out of PSUM)
                    nc.vector.tensor_copy(out=u_b[:], in_=sq[:])

        # combine: Z_b = sum_i P_a[i,b] * u_b[i,b]; denom = ln Z + lnacc
        ab = crfs.tile([L, BL], f32, tag="ab", name="ab")
        nc.vector.tensor_tensor(out=ab[:], in0=P_a[:], in1=u_b[:], op=ALU.mult)
        zrow = bulkp.tile([L, BL], f32, tag="pt", name="zrow")
        nc.tensor.matmul(zrow[0:1, :], lhsT=ones9[:, 0:1], rhs=ab[:],
                         start=True, stop=True)
        lnz = crfs.tile([1, BL], f32, tag="lns", name="lnz")
        nc.scalar.activation(lnz[:], zrow[0:1, :], ACTF.Ln)
        nc.vector.tensor_tensor(out=lnz[:], in0=lnz[:], in1=lnacc[:],
                                op=ALU.add)
        dsc = crfs.tile([1, 1], f32, tag="dsc", name="dsc")
        nc.vector.tensor_reduce(dsc[:], lnz[:], axis=mybir.AxisListType.X,
                                op=ALU.add)
        nc.vector.tensor_reduce(num9[:], num9c[:], axis=mybir.AxisListType.X,
                                op=ALU.add)
        npsum = bulkp.tile([L, BL], f32, tag="pt", name="npsum")
        nc.tensor.matmul(npsum[0:1, 0:1], lhsT=ones9[:, 0:1], rhs=num9[:, 0:1],
                         start=True, stop=True)
        out_sb = crfs.tile([1, 2], f32, tag="out_sb", name="out_sb")
        nc.vector.tensor_scalar(out=out_sb[:, 0:1], in0=npsum[0:1, 0:1],
                                scalar1=0.0, scalar2=None, op0=ALU.add)
        nc.vector.tensor_scalar(out=out_sb[:, 1:2], in0=dsc[:],
                                scalar1=0.0, scalar2=None, op0=ALU.add)
        nc.sync.dma_start(res[:], out_sb[:])


# ---------------------------------------------------------------------------
# host side
# ---------------------------------------------------------------------------

def _prescale(w_ih, w_hh, b_ih, b_hh, h_in_doubled):
    """Gate-slot layout is torch order (i,f,g,o). Returns fp32 arrays."""
    sg = np.ones((4, 1), np.float32)
    sg[2] = 2.0                       # g gate rows x2 (tanh scale 0.5 trick)
    srows = np.repeat(sg, H, axis=0)  # [512, 1]
    wih = w_ih.astype(np.float32) * srows
    whh_ = w_hh.astype(np.float32) * srows * 0.5
    b = (b_ih + b_hh).astype(np.float32) * srows[:, 0]
    if h_in_doubled:
        wih = wih * 0.5
    return wih, whh_, b


GATE_ORDER = (3, 0, 1, 2)   # device slot s holds torch gate GATE_ORDER[s]: o,i,f,g


def _lhsT_gate_major(w, kchunks):
    """w: [4H, K] fp32 -> [nchunk, 128, 512] bf16 lhsT (zero-padded K)."""
    outs = []
    off = 0
    for kk in kchunks:
        blk = np.zeros((128, 512), np.float32)
        take = min(kk, w.shape[1] - off)
        for slot, g in enumerate(GATE_ORDER):
            blk[:take, slot * 128:(slot + 1) * 128] = \
                w[g * H:(g + 1) * H, off:off + take].T
        outs.append(blk)
        off += kk
    return np.stack(outs).astype(np.dtype("bfloat16"))


_PROG_CACHE = {}


def _get_program(Tsteps):
    if Tsteps not in _PROG_CACHE:
        _PROG_CACHE[Tsteps] = build_program(Tsteps)
    return _PROG_CACHE[Tsteps]


def prepare_inputs(inputs, Tsteps=T):
    """Build the per-core input maps + the host numerator constants."""
    bfl = np.dtype("bfloat16")
    words = np.asarray(inputs["word_batch"]).astype(np.int64)
    labels = np.asarray(inputs["label_batch"]).astype(np.int64)
    emb = np.asarray(inputs["emb"], np.float32)
    words = words[:, :Tsteps]
    labels = labels[:, :Tsteps]

    embp = np.zeros((V, DPAD), np.float32)
    embp[:, :D] = emb
    embp = embp.astype(bfl)

    ident = np.eye(128, dtype=np.float32).astype(bfl)

    wih0_l, whh_l, wih1_l, bias_l = [], [], [], []
    for layer, (wihk, whhk, bihk, bhhk) in enumerate(
            [("w_ih_l0", "w_hh_l0", "b_ih_l0", "b_hh_l0"),
             ("w_ih_l1", "w_hh_l1", "b_ih_l1", "b_hh_l1")]):
        for d in range(2):
            wih, whh_, b = _prescale(
                np.asarray(inputs[wihk])[d], np.asarray(inputs[whhk])[d],
                np.asarray(inputs[bihk])[d], np.asarray(inputs[bhhk])[d],
                h_in_doubled=(layer == 1))
            if layer == 0:
                wihp = np.zeros((512, DPAD), np.float32)
                wihp[:, :D] = wih
                wih0_l.append(_lhsT_gate_major(wihp, KCH))
            else:
                wih1_l.append(_lhsT_gate_major(wih, (128, 128)))
            whh_l.append(_lhsT_gate_major(whh_, (128,)))
            bias_l.append(b.reshape(4, H)[list(GATE_ORDER)].T)  # [128, 4]
    wih0 = np.stack(wih0_l)                       # [2, 3, 128, 512]
    wih1 = np.stack(wih1_l)                       # [2, 2, 128, 512]
    whh = np.stack(whh_l).reshape(2, 2, 1, 128, 512)[:, :, 0]
    biases = np.stack(bias_l).reshape(2, 2, 128, 4).astype(np.float32)

    w_out = np.asarray(inputs["w_out"], np.float32) * 0.5   # [L, 2H]
    woutT = np.stack([w_out[:, :H].T, w_out[:, H:].T]).astype(bfl)  # [2,128,L]
    boutp = (np.asarray(inputs["b_out"], np.float32) - CBAR).reshape(L, 1)
    estart = np.exp(np.asarray(inputs["start_t"], np.float32)).reshape(L, 1)
    expEm = np.exp(np.asarray(inputs["trans"], np.float32))
    expend = np.exp(np.asarray(inputs["end_t"], np.float32)).reshape(L, 1)

    NT = Tsteps * BL
    in_maps = []
    num_consts = []
    start_t = np.asarray(inputs["start_t"], np.float32)
    end_t = np.asarray(inputs["end_t"], np.float32)
    trans = np.asarray(inputs["trans"], np.float32)
    bshift = np.asarray(inputs["b_out"], np.float32) - CBAR   # device em offset
    for c in range(NCORES):
        bs = slice(c * BL, (c + 1) * BL)
        wc = words[bs]                        # [BL, Tsteps]
        lc = labels[bs]
        toks = wc.T.reshape(-1).astype(np.int32)          # (t, b) order
        ohc = (lc.T.reshape(1, -1) == np.arange(L).reshape(L, 1))
        in_maps.append({
            "words": np.ascontiguousarray(toks.reshape(NT // 128, 128).T),
            "emb": embp, "ident": ident,
            "wih0": wih0, "wih1": wih1, "whh": whh, "biases": biases,
            "woutT": woutT, "bout": boutp, "estart": estart,
            "expE": expEm, "expET": np.ascontiguousarray(expEm.T),
            "expend": expend,
            "oh": ohc.astype(bfl),
        })
        num_consts.append(
            float(start_t[lc[:, 0]].sum())
            + float(trans[lc[:, :-1], lc[:, 1:]].sum())
            + float(end_t[lc[:, -1]].sum())
            + float(bshift[lc].sum()))        # device num omits the em bias
    return in_maps, num_consts


def kernel(**inputs):
    in_maps, num_consts = prepare_inputs(inputs, T)
    nc = _get_program(T)
    out = run_bass_kernel_spmd(nc, in_maps, list(range(NCORES)))
    llh = 0.0
    for c in range(NCORES):
        r = out.results[c]["res"].reshape(2).astype(np.float64)
        llh += num_consts[c] + r[0] - r[1]
    return np.float32(-llh)


if __name__ == "__main__":
    np.random.seed(0)
    print("building program (small T) ...")
    build_program(16)
    print("ok")


# revision 27
# speedup vs baseline: 1.0163x; 1.0001x over previous
"""BiLSTM-CRF forward NLL on 8 Trainium2 NeuronCores.

Sharding: pure data-parallel over batch (8 sequences per core), params
replicated. Per core: embedding gather -> bulk input matmuls -> 2-layer
BiLSTM recurrence (fwd/bwd chains interleaved per layer) -> emissions ->
CRF scan -> partial (num, denom) pair. Host sums partials with the
label-dependent numerator constant.

Key restructurings (validated against the reference):
  * LSTM cell uses a single tanh activation per step over all 4 gates:
    sigmoid(x) = (tanh(x/2)+1)/2, with the tanh(0.5*...) instruction scale
    folded into host-prescaled weights (g-gate rows x2). The cell tracks
    C = 2c and H = 2h; every consumer of h (recurrent weights, layer-1
    input weights, output projection) is pre-halved on the host.
  * Gate slots are laid out (o,i,f,g) with a per-dir persistent cell tile
    [o i f g C] so one fused DVE op computes v2=(yi+1)*yg and z=(yf+1)*C.
  * All tile pools coexist (PSUM banks budgeted to 8) so the readiness-
    driven scheduler overlaps: gather with bulk-0 (gather emitted from
    both sequence ends, bulk-0 bwd chunks reversed), bulk-1 with the
    layer-0 recurrence (h0 becomes available middle-out), and emissions
    (+exp +numerator partials) with the layer-1 recurrence.
  * CRF runs in probability space, split into a forward alpha chain
    (t=0..255) and a backward beta chain (t=511..256) that execute
    concurrently, halving the sequential CRF depth. State ping-pongs
    between PSUM (matmul out) and SBUF (elementwise mult with expem),
    with one ln-renorm per chain. em' = em + b_out - log(L); the log(L)
    shift cancels between numerator and denominator.
"""

import os
import sys

import numpy as np

sys.path.insert(0, "/opt/trn_rl_repo")

import concourse.bass as bass
import concourse.tile as tile
from concourse import bacc, mybir
from concourse.bass_utils import run_bass_kernel_spmd

B, T, V, D, HD, L = 64, 512, 100000, 300, 256, 9
H = 128
NCORES = 8
BL = B // NCORES          # sequences per core
DPAD = 384                # D padded so DMA-transpose chunks are 128 wide
KCH = (128, 128, 128)     # K chunks of DPAD
CBAR = float(np.log(L))   # per-step CRF shift (cancels in num - denom)

f32 = mybir.dt.float32
bf16 = mybir.dt.bfloat16
i32 = mybir.dt.int32
ALU = mybir.AluOpType
ACTF = mybir.ActivationFunctionType


# ---------------------------------------------------------------------------
# device program
# ---------------------------------------------------------------------------

def build_program(Tsteps=T):
    NT = Tsteps * BL
    NCK = max(1, NT // 512)            # bulk matmul N chunks
    NCOLS = NT // NCK
    NTILES = NT // 128                 # gather tiles
    assert NT % 128 == 0 and NT % NCK == 0

    nc = bacc.Bacc("TRN2", target_bir_lowering=False, debug=False)

    def din(name, shape, dt):
        return nc.dram_tensor(name, shape, dt, kind="ExternalInput").ap()

    words = din("words", [128, NTILES], i32)
    emb = din("emb", [V, DPAD], bf16)
    ident = din("ident", [128, 128], bf16)
    # lhsT weights, gate-major free dim (slots o,i,f,g each 128 wide)
    wih0 = din("wih0", [2, 3, 128, 512], bf16)     # [dir][kchunk][K][4*128]
    wih1 = din("wih1", [2, 2, 128, 512], bf16)     # [dir][h0-dir kchunk][K][4*128]
    whh = din("whh", [2, 2, 128, 512], bf16)       # [layer][dir][K=H][4*128]
    biases = din("biases", [2, 2, 128, 4], f32)    # [layer][dir][hidden][gate]
    woutT = din("woutT", [2, 128, L], bf16)        # [h1-dir kchunk][K][L]
    bout = din("bout", [L, 1], f32)                # b_out - CBAR
    estart = din("estart", [L, 1], f32)            # exp(start_t)
    expE = din("expE", [L, L], f32)                # exp(trans)
    expET = din("expET", [L, L], f32)              # exp(trans).T
    expend = din("expend", [L, 1], f32)            # exp(end_t)
    oh = din("oh", [L, NT], bf16)                  # label one-hot, (t,b) order
    res = nc.dram_tensor("res", [1, 2], f32, kind="ExternalOutput").ap()

    with tile.TileContext(nc) as tc:
        _emit(tc, nc, Tsteps, NT, NCK, NCOLS, NTILES,
              words, emb, ident, wih0, wih1, whh, biases, woutT, bout,
              estart, expE, expET, expend, oh, res)
    nc.compile()
    return nc


def _emit(tc, nc, Tsteps, NT, NCK, NCOLS, NTILES,
          words, emb, ident, wih0, wih1, whh, biases, woutT, bout,
          estart, expE, expET, expend, oh, res):
    from contextlib import ExitStack

    TM = Tsteps // 2            # alpha covers t<=TM-1... (split point)
    RENORM_K = 128              # renorm once per chain at this chain-step

    ctx = ExitStack()
    with ctx:
        consts = ctx.enter_context(tc.tile_pool(name="consts", bufs=1))
        states = ctx.enter_context(tc.tile_pool(name="states", bufs=1))
        # PSUM budget (8 banks, bank-granular per tag-slot):
        #   bulkp 1 tag x 2 bufs + gatesp 2 tags x 2 bufs + tpp 1 tag x 2.
        # CRF alpha/beta matmul tiles reuse tpp's tag (disjoint phases).
        bulkp = ctx.enter_context(
            tc.tile_pool(name="bulkp", bufs=2, space="PSUM"))
        gatesp = ctx.enter_context(
            tc.tile_pool(name="gatesp", bufs=2, space="PSUM"))
        tpp = ctx.enter_context(
            tc.tile_pool(name="tpp", bufs=2, space="PSUM"))
        crfp = tpp
        wpool = ctx.enter_context(tc.tile_pool(name="wpool", bufs=4))
        scrp = ctx.enter_context(tc.tile_pool(name="scrp", bufs=2))
        crfs = ctx.enter_context(tc.tile_pool(name="crfs", bufs=4))
        xTp = ctx.enter_context(tc.tile_pool(name="xT", bufs=1))

        # ---- persistent SBUF tiles ----
        ident_sb = consts.tile([128, 128], bf16, tag="ident")
        nc.sync.dma_start(ident_sb[:], ident[:])
        whh_sb = {}
        for l in range(2):
            for d in range(2):
                t_ = consts.tile([128, 512], bf16, name=f"whh{l}{d}")
                nc.sync.dma_start(t_[:], whh[l, d])
                whh_sb[l, d] = t_
        bias_sb = {}
        for l in range(2):
            for d in range(2):
                t_ = consts.tile([128, 4], f32, name=f"bias{l}{d}")
                nc.sync.dma_start(t_[:], biases[l, d])
                bias_sb[l, d] = t_
        wih0_sb = {}
        for d in range(2):
            for c in range(3):
                t_ = consts.tile([128, 512], bf16, name=f"wih0_{d}{c}")
                nc.sync.dma_start(t_[:], wih0[d, c])
                wih0_sb[d, c] = t_
        wih1_sb = {}
        for d in range(2):
            for k in range(2):
                t_ = consts.tile([128, 512], bf16, name=f"wih1_{d}{k}")
                nc.sync.dma_start(t_[:], wih1[d, k])
                wih1_sb[d, k] = t_
        woutT_sb = [consts.tile([128, L], bf16, name=f"wo{k}") for k in range(2)]
        for k in range(2):
            nc.sync.dma_start(woutT_sb[k][:], woutT[k])
        bout_sb = consts.tile([L, 1], f32, name="bout_sb")
        estart_sb = consts.tile([L, 1], f32, name="estart_sb")
        expE_sb = consts.tile([L, L], f32, name="expE_sb")
        expET_sb = consts.tile([L, L], f32, name="expET_sb")
        expend_sb = consts.tile([L, 1], f32, name="expend_sb")
        ones9 = consts.tile([L, L], f32, name="ones9")
        nc.sync.dma_start(bout_sb[:], bout[:])
        nc.sync.dma_start(estart_sb[:], estart[:])
        nc.sync.dma_start(expE_sb[:], expE[:])
        nc.sync.dma_start(expET_sb[:], expET[:])
        nc.sync.dma_start(expend_sb[:], expend[:])
        nc.vector.memset(ones9[:], 1.0)

        h_hist = {}
        for l in range(2):
            for d in range(2):
                h_hist[l, d] = states.tile([128, NT], bf16, name=f"h{l}{d}")
        # per-dir persistent cell tile, cols [o i f g C] (x BL each); the
        # C slot makes the fused (v2|z) DVE op's in1 = [g C] contiguous.
        ycell = [states.tile([128, 5 * BL], f32, name=f"yc{d}") for d in range(2)]
        tcl_st = [states.tile([128, BL], f32, name=f"tcl{d}") for d in range(2)]

        # xp for the two directions of the current layer (reused across layers)
        xp_sb = [states.tile([128, 4 * NT], bf16, name=f"xp{d}") for d in range(2)]

        # CRF persistent tiles
        expem = states.tile([L, NT], f32, name="expem")
        oh_sb = states.tile([L, NT], bf16, name="oh_sb")
        nc.sync.dma_start(oh_sb[:], oh[:])
        P_a = states.tile([L, BL], f32, name="P_a")       # alpha (SBUF leg)
        u_b = states.tile([L, BL], f32, name="u_b")       # beta (SBUF leg)
        lnacc = states.tile([1, BL], f32, name="lnacc")
        num9c = states.tile([L, NCK], f32, name="num9c")
        num9 = states.tile([L, 1], f32, name="num9")
        nc.vector.memset(lnacc[:], 0.0)

        def bulk_chunk(layer, d, nck, srcs):
            """xp[d][:, chunk nck] = srcs-matmul + bias, gate slots (o,i,f,g)."""
            nsl = slice(nck * NCOLS, (nck + 1) * NCOLS)
            xv = xp_sb[d][:].rearrange("p (t g b) -> p t g b", g=4, b=BL)
            tpc = NCOLS // BL
            for slot in range(4):
                pt = bulkp.tile([128, NCOLS], f32, tag="pt", name="pt")
                for ki, (src, wt, kk) in enumerate(srcs):
                    nc.tensor.matmul(
                        pt[:],
                        lhsT=wt[:kk, slot * 128:(slot + 1) * 128],
                        rhs=src[:kk, nsl],
                        start=(ki == 0), stop=(ki == len(srcs) - 1),
                    )
                nc.vector.tensor_scalar(
                    out=xv[:, nck * tpc:(nck + 1) * tpc, slot, :],
                    in0=pt[:].rearrange("p (t b) -> p t b", b=BL),
                    scalar1=bias_sb[layer, d][:, slot:slot + 1],
                    scalar2=None, op0=ALU.add,
                )

        def recur_phase(layer):
            # Two independent dir-chains; each cell is PE -> ACT -> DVE(w)
            # -> DVE(C) -> ACT -> DVE(h). Wall time = T x chain path; the
            # two chains overlap on the engines.
            for d in range(2):
                nc.vector.memset(ycell[d][:, 4 * BL:5 * BL], 0.0)
            for t in range(Tsteps):
                taus = (t, Tsteps - 1 - t)
                first = (t == 0)
                for d in range(2):
                    tau = taus[d]
                    y = ycell[d]
                    gp = gatesp.tile([128, 4 * BL], f32, tag=f"gp{d}",
                                     name=f"gp{d}")
                    nc.tensor.matmul(gp[:], lhsT=ident_sb[:],
                                     rhs=xp_sb[d][:, tau * 4 * BL:(tau + 1) * 4 * BL],
                                     start=True, stop=first)
                    if not first:
                        prev = tau - 1 if d == 0 else tau + 1
                        hh = h_hist[layer, d]
                        whh_t = whh_sb[layer, d]
                        for slot in range(4):
                            nc.tensor.matmul(
                                gp[:, slot * BL:(slot + 1) * BL],
                                lhsT=whh_t[:, slot * 128:(slot + 1) * 128],
                                rhs=hh[:, prev * BL:(prev + 1) * BL],
                                start=False, stop=(slot == 3))
                    # y[0:4BL] = tanh(gates/2), slots (o,i,f,g)
                    nc.scalar.activation(y[:, 0:4 * BL], gp[:], ACTF.Tanh,
                                         scale=0.5)
                    # w = [(yi+1)*yg | (yf+1)*C_old] = [v2 | z]
                    w = wpool.tile([128, 2 * BL], f32, tag=f"w{d}", name=f"w{d}")
                    nc.vector.scalar_tensor_tensor(
                        w[:], in0=y[:, BL:3 * BL], scalar=1.0,
                        in1=y[:, 3 * BL:5 * BL], op0=ALU.add, op1=ALU.mult)
                    # C = 0.5*z + v2
                    nc.vector.scalar_tensor_tensor(
                        y[:, 4 * BL:5 * BL], in0=w[:, BL:2 * BL], scalar=0.5,
                        in1=w[:, 0:BL], op0=ALU.mult, op1=ALU.add)
                    nc.scalar.activation(tcl_st[d][:], y[:, 4 * BL:5 * BL],
                                         ACTF.Tanh, scale=0.5)
                    nc.vector.scalar_tensor_tensor(
                        h_hist[layer, d][:, tau * BL:(tau + 1) * BL],
                        in0=y[:, 0:BL], scalar=1.0, in1=tcl_st[d][:],
                        op0=ALU.add, op1=ALU.mult)

        # =================================================================
        # Phase 1: embedding gather + transpose (emitted from both sequence
        # ends so both bulk-0 dir-chunk streams start early)
        # =================================================================
        x_T = [xTp.tile([k, NT], bf16, name=f"xT{c}") for c, k in enumerate(KCH)]
        idx_all = consts.tile([128, NTILES], i32, name="idx_all")
        nc.sync.dma_start(idx_all[:], words[:])
        with tc.tile_pool(name="gath", bufs=4) as gp_:
            order = []
            lo, hi = 0, NTILES - 1
            while lo <= hi:
                order.append(lo)
                if hi != lo:
                    order.append(hi)
                lo, hi = lo + 1, hi - 1
            for i in order:
                g = gp_.tile([128, DPAD], bf16, tag="g", name="g")
                nc.gpsimd.indirect_dma_start(
                    out=g[:], out_offset=None, in_=emb[:],
                    in_offset=bass.IndirectOffsetOnAxis(ap=idx_all[:, i:i + 1],
                                                        axis=0),
                )
                # transpose on the (idle) PE + DVE copy-back: keeps the
                # HWDGE free so it never stalls the frozen PE stream.
                for c, k in enumerate(KCH):
                    tp = tpp.tile([128, 128], bf16, tag="tp", name="tp")
                    nc.tensor.transpose(tp[:], g[:, c * 128:(c + 1) * 128],
                                        ident_sb[:])
                    nc.vector.tensor_copy(out=x_T[c][:, i * 128:(i + 1) * 128],
                                          in_=tp[:])

        # bulk-0: fwd chunks ascending, bwd chunks descending, interleaved.
        # Deprioritized so the scheduler runs recurrence ops first in any
        # engine-idle gap (bulk fills the slack; data deps still hold).
        LOW = -10_000_000
        srcs0 = lambda d: [(x_T[c], wih0_sb[d, c], KCH[c]) for c in range(3)]
        with tc.high_priority(offset=LOW):
            for j in range(NCK):
                bulk_chunk(0, 0, j, srcs0(0))
                bulk_chunk(0, 1, NCK - 1 - j, srcs0(1))

        recur_phase(0)

        # bulk-1 middle-out: h0 regions complete middle-out during recur 0,
        # so these run concurrently with the tail of the layer-0 recurrence.
        srcs1 = lambda d: [(h_hist[0, k], wih1_sb[d, k], 128) for k in range(2)]
        mid_order = []
        lo, hi = NCK // 2 - 1, NCK // 2
        while lo >= 0:
            mid_order.extend([hi, lo])
            lo, hi = lo - 1, hi + 1
        with tc.high_priority(offset=LOW):
            for j in mid_order:
                for d in range(2):
                    bulk_chunk(1, d, j, srcs1(d))

        recur_phase(1)

        # =================================================================
        # Emissions (middle-out, overlap recur 1): per chunk
        #   pt = w_out @ h1 (PSUM); expem = exp(pt + bout') (ACT);
        #   num partial = sum(pt * onehot) via accum_out (DVE)
        # =================================================================
        with tc.high_priority(offset=LOW):
            for nck in mid_order:
                nsl = slice(nck * NCOLS, (nck + 1) * NCOLS)
                pt = bulkp.tile([L, NCOLS], f32, tag="pt", name="pt")
                for k in range(2):
                    nc.tensor.matmul(pt[:], lhsT=woutT_sb[k][:],
                                     rhs=h_hist[1, k][:, nsl],
                                     start=(k == 0), stop=(k == 1))
                nc.scalar.activation(expem[:, nsl], pt[:], ACTF.Exp,
                                     bias=bout_sb[:, 0:1])
                scr = scrp.tile([L, NCOLS], f32, tag="scr", name="scr")
                nc.vector.scalar_tensor_tensor(
                    scr[:], in0=pt[:], scalar=0.0, in1=oh_sb[:, nsl],
                    op0=ALU.add, op1=ALU.mult,
                    accum_out=num9c[:, nck:nck + 1])

        # =================================================================
        # CRF: two-ended scan in probability space.
        #   alpha: P_{t} = (E^T P_{t-1}) o expem_t   for t = 1..TM-1
        #   beta:  Q_{t} = E (expem_{t+1} o Q_{t+1}) for t = T-2..TM-1
        #   Z_b = sum_i alpha_{TM-1}[i] * beta_{TM-1}[i]
        # State ping-pongs PSUM (matmul) <-> SBUF (mult); one renorm each.
        # =================================================================
        def renorm(vec_sb, b_lnacc_col):
            srow = bulkp.tile([L, BL], f32, tag="pt", name="srow")
            nc.tensor.matmul(srow[:], lhsT=ones9[:], rhs=vec_sb[:],
                             start=True, stop=True)
            lns = crfs.tile([1, BL], f32, tag="lns", name="lns")
            nc.scalar.activation(lns[:], srow[0:1, :], ACTF.Ln)
            nc.vector.tensor_tensor(out=lnacc[:], in0=lnacc[:], in1=lns[:],
                                    op=ALU.add)
            rec = crfs.tile([L, BL], f32, tag="rec", name="rec")
            nc.vector.reciprocal(rec[:], srow[:])
            nc.vector.tensor_tensor(out=vec_sb[:], in0=vec_sb[:], in1=rec[:],
                                    op=ALU.mult)

        # alpha init: P_0 = exp(start) o expem_0
        nc.vector.tensor_scalar(out=P_a[:], in0=expem[:, 0:BL],
                                scalar1=estart_sb[:, 0:1], scalar2=None,
                                op0=ALU.mult)
        # beta init: u = expem_{T-1} o expend  (u for producing Q_{T-2})
        nc.vector.tensor_scalar(out=u_b[:], in0=expem[:, (Tsteps - 1) * BL:],
                                scalar1=expend_sb[:, 0:1], scalar2=None,
                                op0=ALU.mult)
        # chains: alpha consumes expem_1..expem_{TM-1};
        # beta consumes expem_{T-2}..expem_{TM} then combines at TM-1.
        n_a = TM - 1                 # alpha steps (t = 1..TM-1)
        n_b = Tsteps - TM            # beta matmul steps producing Q_{TM-1}
        for k in range(1, max(n_a, n_b) + 1):
            if k <= n_a:
                sp = crfp.tile([L, BL], f32, tag="tp", name="sp")
                nc.tensor.matmul(sp[:], lhsT=expE_sb[:], rhs=P_a[:],
                                 start=True, stop=True)
                nc.vector.tensor_tensor(
                    out=P_a[:], in0=sp[:],
                    in1=expem[:, k * BL:(k + 1) * BL], op=ALU.mult)
                if k == RENORM_K:
                    renorm(P_a, 0)
            if k <= n_b:
                sq = crfp.tile([L, BL], f32, tag="tp", name="sq")
                nc.tensor.matmul(sq[:], lhsT=expET_sb[:], rhs=u_b[:],
                                 start=True, stop=True)
                t_next = Tsteps - 1 - k      # Q_{t_next} now in sq
                if k < n_b:
                    nc.vector.tensor_tensor(
                        out=u_b[:], in0=sq[:],
                        in1=expem[:, t_next * BL:(t_next + 1) * BL],
                        op=ALU.mult)
                    if k == RENORM_K:
                        renorm(u_b, 0)
                else:
                    # final: u_b <- Q_{TM-1} (plain copy out of PSUM)
                    nc.vector.tensor_copy(out=u_b[:], in_=sq[:])

        # combine: Z_b = sum_i P_a[i,b] * u_b[i,b]; denom = ln Z + lnacc
        ab = crfs.tile([L, BL], f32, tag="ab", name="ab")
        nc.vector.tensor_tensor(out=ab[:], in0=P_a[:], in1=u_b[:], op=ALU.mult)
        zrow = bulkp.tile([L, BL], f32, tag="pt", name="zrow")
        nc.tensor.matmul(zrow[0:1, :], lhsT=ones9[:, 0:1], rhs=ab[:],
                         start=True, stop=True)
        lnz = crfs.tile([1, BL], f32, tag="lns", name="lnz")
        nc.scalar.activation(lnz[:], zrow[0:1, :], ACTF.Ln)
        nc.vector.tensor_tensor(out=lnz[:], in0=lnz[:], in1=lnacc[:],
                                op=ALU.add)
        dsc = crfs.tile([1, 1], f32, tag="dsc", name="dsc")
        nc.vector.tensor_reduce(dsc[:], lnz[:], axis=mybir.AxisListType.X,
                                op=ALU.add)
        nc.vector.tensor_reduce(num9[:], num9c[:], axis=mybir.AxisListType.X,
                                op=ALU.add)
        npsum = bulkp.tile([L, BL], f32, tag="pt", name="npsum")
        nc.tensor.matmul(npsum[0:1, 0:1], lhsT=ones9[:, 0:1], rhs=num9[:, 0:1],
                         start=True, stop=True)
        out_sb = crfs.tile([1, 2], f32, tag="out_sb", name="out_sb")
        nc.vector.tensor_scalar(out=out_sb[:, 0:1], in0=npsum[0:1, 0:1],
                                scalar1=0.0, scalar2=None, op0=ALU.add)
        nc.vector.tensor_scalar(out=out_sb[:, 1:2], in0=dsc[:],
                                scalar1=0.0, scalar2=None, op0=ALU.add)
        nc.sync.dma_start(res[:], out_sb[:])


# ---------------------------------------------------------------------------
# host side
# ---------------------------------------------------------------------------

def _prescale(w_ih, w_hh, b_ih, b_hh, h_in_doubled):
    """Gate-slot layout is torch order (i,f,g,o). Returns fp32 arrays."""
    sg = np.ones((4, 1), np.float32)
    sg[2] = 2.0                       # g gate rows x2 (tanh scale 0.5 trick)
    srows = np.repeat(sg, H, axis=0)  # [512, 1]
    wih = w_ih.astype(np.float32) * srows
    whh_ = w_hh.astype(np.float32) * srows * 0.5
    b = (b_ih + b_hh).astype(np.float32) * srows[:, 0]
    if h_in_doubled:
        wih = wih * 0.5
    return wih, whh_, b


GATE_ORDER = (3, 0, 1, 2)   # device slot s holds torch gate GATE_ORDER[s]: o,i,f,g


def _lhsT_gate_major(w, kchunks):
    """w: [4H, K] fp32 -> [nchunk, 128, 512] bf16 lhsT (zero-padded K)."""
    outs = []
    off = 0
    for kk in kchunks:
        blk = np.zeros((128, 512), np.float32)
        take = min(kk, w.shape[1] - off)
        for slot, g in enumerate(GATE_ORDER):
            blk[:take, slot * 128:(slot + 1) * 128] = \
                w[g * H:(g + 1) * H, off:off + take].T
        outs.append(blk)
        off += kk
    return np.stack(outs).astype(np.dtype("bfloat16"))


_PROG_CACHE = {}


def _get_program(Tsteps):
    if Tsteps not in _PROG_CACHE:
        _PROG_CACHE[Tsteps] = build_program(Tsteps)
    return _PROG_CACHE[Tsteps]


def prepare_inputs(inputs, Tsteps=T):
    """Build the per-core input maps + the host numerator constants."""
    bfl = np.dtype("bfloat16")
    words = np.asarray(inputs["word_batch"]).astype(np.int64)
    labels = np.asarray(inputs["label_batch"]).astype(np.int64)
    emb = np.asarray(inputs["emb"], np.float32)
    words = words[:, :Tsteps]
    labels = labels[:, :Tsteps]

    embp = np.zeros((V, DPAD), np.float32)
    embp[:, :D] = emb
    embp = embp.astype(bfl)

    ident = np.eye(128, dtype=np.float32).astype(bfl)

    wih0_l, whh_l, wih1_l, bias_l = [], [], [], []
    for layer, (wihk, whhk, bihk, bhhk) in enumerate(
            [("w_ih_l0", "w_hh_l0", "b_ih_l0", "b_hh_l0"),
             ("w_ih_l1", "w_hh_l1", "b_ih_l1", "b_hh_l1")]):
        for d in range(2):
            wih, whh_, b = _prescale(
                np.asarray(inputs[wihk])[d], np.asarray(inputs[whhk])[d],
                np.asarray(inputs[bihk])[d], np.asarray(inputs[bhhk])[d],
                h_in_doubled=(layer == 1))
            if layer == 0:
                wihp = np.zeros((512, DPAD), np.float32)
                wihp[:, :D] = wih
                wih0_l.append(_lhsT_gate_major(wihp, KCH))
            else:
                wih1_l.append(_lhsT_gate_major(wih, (128, 128)))
            whh_l.append(_lhsT_gate_major(whh_, (128,)))
            bias_l.append(b.reshape(4, H)[list(GATE_ORDER)].T)  # [128, 4]
    wih0 = np.stack(wih0_l)                       # [2, 3, 128, 512]
    wih1 = np.stack(wih1_l)                       # [2, 2, 128, 512]
    whh = np.stack(whh_l).reshape(2, 2, 1, 128, 512)[:, :, 0]
    biases = np.stack(bias_l).reshape(2, 2, 128, 4).astype(np.float32)

    w_out = np.asarray(inputs["w_out"], np.float32) * 0.5   # [L, 2H]
    woutT = np.stack([w_out[:, :H].T, w_out[:, H:].T]).astype(bfl)  # [2,128,L]
    boutp = (np.asarray(inputs["b_out"], np.float32) - CBAR).reshape(L, 1)
    estart = np.exp(np.asarray(inputs["start_t"], np.float32)).reshape(L, 1)
    expEm = np.exp(np.asarray(inputs["trans"], np.float32))
    expend = np.exp(np.asarray(inputs["end_t"], np.float32)).reshape(L, 1)

    NT = Tsteps * BL
    in_maps = []
    num_consts = []
    start_t = np.asarray(inputs["start_t"], np.float32)
    end_t = np.asarray(inputs["end_t"], np.float32)
    trans = np.asarray(inputs["trans"], np.float32)
    bshift = np.asarray(inputs["b_out"], np.float32) - CBAR   # device em offset
    for c in range(NCORES):
        bs = slice(c * BL, (c + 1) * BL)
        wc = words[bs]                        # [BL, Tsteps]
        lc = labels[bs]
        toks = wc.T.reshape(-1).astype(np.int32)          # (t, b) order
        ohc = (lc.T.reshape(1, -1) == np.arange(L).reshape(L, 1))
        in_maps.append({
            "words": np.ascontiguousarray(toks.reshape(NT // 128, 128).T),
            "emb": embp, "ident": ident,
            "wih0": wih0, "wih1": wih1, "whh": whh, "biases": biases,
            "woutT": woutT, "bout": boutp, "estart": estart,
            "expE": expEm, "expET": np.ascontiguousarray(expEm.T),
            "expend": expend,
            "oh": ohc.astype(bfl),
        })
        num_consts.append(
            float(start_t[lc[:, 0]].sum())
            + float(trans[lc[:, :-1], lc[:, 1:]].sum())
            + float(end_t[lc[:, -1]].sum())
            + float(bshift[lc].sum()))        # device num omits the em bias
    return in_maps, num_consts


def kernel(**inputs):
    in_maps, num_consts = prepare_inputs(inputs, T)
    nc = _get_program(T)
    out = run_bass_kernel_spmd(nc, in_maps, list(range(NCORES)))
    llh = 0.0
    for c in range(NCORES):
        r = out.results[c]["res"].reshape(2).astype(np.float64)
        llh += num_consts[c] + r[0] - r[1]
    return np.float32(-llh)


if __name__ == "__main__":
    np.random.seed(0)
    print("building program (small T) ...")
    build_program(16)
    print("ok")


# revision 28
# speedup vs baseline: 1.0281x; 1.0116x over previous
"""BiLSTM-CRF forward NLL on 8 Trainium2 NeuronCores.

Sharding: pure data-parallel over batch (8 sequences per core), params
replicated. Per core: embedding gather -> bulk input matmuls -> 2-layer
BiLSTM recurrence (fwd/bwd chains interleaved per layer) -> emissions ->
CRF scan -> partial (num, denom) pair. Host sums partials with the
label-dependent numerator constant.

Key restructurings (validated against the reference):
  * LSTM cell uses a single tanh activation per step over all 4 gates:
    sigmoid(x) = (tanh(x/2)+1)/2, with the tanh(0.5*...) instruction scale
    folded into host-prescaled weights (g-gate rows x2). The cell tracks
    C = 2c and H = 2h; every consumer of h (recurrent weights, layer-1
    input weights, output projection) is pre-halved on the host.
  * Gate slots are laid out (o,i,f,g) with a per-dir persistent cell tile
    [o i f g C] so one fused DVE op computes v2=(yi+1)*yg and z=(yf+1)*C.
  * All tile pools coexist (PSUM banks budgeted to 8) so the readiness-
    driven scheduler overlaps: gather with bulk-0 (gather emitted from
    both sequence ends, bulk-0 bwd chunks reversed), bulk-1 with the
    layer-0 recurrence (h0 becomes available middle-out), and emissions
    (+exp +numerator partials) with the layer-1 recurrence.
  * CRF runs in probability space, split into a forward alpha chain
    (t=0..255) and a backward beta chain (t=511..256) that execute
    concurrently, halving the sequential CRF depth. State ping-pongs
    between PSUM (matmul out) and SBUF (elementwise mult with expem),
    with one ln-renorm per chain. em' = em + b_out - log(L); the log(L)
    shift cancels between numerator and denominator.
"""

import os
import sys

import numpy as np

sys.path.insert(0, "/opt/trn_rl_repo")

import concourse.bass as bass
import concourse.tile as tile
from concourse import bacc, mybir
from concourse.bass_utils import run_bass_kernel_spmd

B, T, V, D, HD, L = 64, 512, 100000, 300, 256, 9
H = 128
NCORES = 8
BL = B // NCORES          # sequences per core
DPAD = 384                # D padded so DMA-transpose chunks are 128 wide
KCH = (128, 128, 128)     # K chunks of DPAD
CBAR = float(np.log(L))   # per-step CRF shift (cancels in num - denom)

f32 = mybir.dt.float32
bf16 = mybir.dt.bfloat16
i32 = mybir.dt.int32
ALU = mybir.AluOpType
ACTF = mybir.ActivationFunctionType


# ---------------------------------------------------------------------------
# device program
# ---------------------------------------------------------------------------

def build_program(Tsteps=T):
    NT = Tsteps * BL
    NCK = max(1, NT // 512)            # bulk matmul N chunks
    NCOLS = NT // NCK
    NTILES = NT // 128                 # gather tiles
    assert NT % 128 == 0 and NT % NCK == 0

    nc = bacc.Bacc("TRN2", target_bir_lowering=False, debug=False)

    def din(name, shape, dt):
        return nc.dram_tensor(name, shape, dt, kind="ExternalInput").ap()

    words = din("words", [128, NTILES], i32)
    emb = din("emb", [V, DPAD], bf16)
    ident = din("ident", [128, 128], bf16)
    # lhsT weights, gate-major free dim (slots o,i,f,g each 128 wide)
    wih0 = din("wih0", [2, 3, 128, 512], bf16)     # [dir][kchunk][K][4*128]
    wih1 = din("wih1", [2, 2, 128, 512], bf16)     # [dir][h0-dir kchunk][K][4*128]
    whh = din("whh", [2, 2, 128, 512], bf16)       # [layer][dir][K=H][4*128]
    biases = din("biases", [2, 2, 128, 4], f32)    # [layer][dir][hidden][gate]
    woutT = din("woutT", [2, 128, L], bf16)        # [h1-dir kchunk][K][L]
    bout = din("bout", [L, 1], f32)                # b_out - CBAR
    estart = din("estart", [L, 1], f32)            # exp(start_t)
    expE = din("expE", [L, L], f32)                # exp(trans)
    expET = din("expET", [L, L], f32)              # exp(trans).T
    expend = din("expend", [L, 1], f32)            # exp(end_t)
    oh = din("oh", [L, NT], bf16)                  # label one-hot, (t,b) order
    res = nc.dram_tensor("res", [1, 2], f32, kind="ExternalOutput").ap()

    with tile.TileContext(nc) as tc:
        _emit(tc, nc, Tsteps, NT, NCK, NCOLS, NTILES,
              words, emb, ident, wih0, wih1, whh, biases, woutT, bout,
              estart, expE, expET, expend, oh, res)
    nc.compile()
    return nc


def _emit(tc, nc, Tsteps, NT, NCK, NCOLS, NTILES,
          words, emb, ident, wih0, wih1, whh, biases, woutT, bout,
          estart, expE, expET, expend, oh, res):
    from contextlib import ExitStack

    TM = Tsteps // 2            # alpha covers t<=TM-1... (split point)
    RENORM_K = 128              # renorm once per chain at this chain-step

    ctx = ExitStack()
    with ctx:
        consts = ctx.enter_context(tc.tile_pool(name="consts", bufs=1))
        states = ctx.enter_context(tc.tile_pool(name="states", bufs=1))
        # PSUM budget (8 banks, bank-granular per tag-slot):
        #   bulkp 1 tag x 2 bufs + gatesp 2 tags x 2 bufs + tpp 1 tag x 2.
        # CRF alpha/beta matmul tiles reuse tpp's tag (disjoint phases).
        bulkp = ctx.enter_context(
            tc.tile_pool(name="bulkp", bufs=2, space="PSUM"))
        gatesp = ctx.enter_context(
            tc.tile_pool(name="gatesp", bufs=2, space="PSUM"))
        tpp = ctx.enter_context(
            tc.tile_pool(name="tpp", bufs=2, space="PSUM"))
        crfp = tpp
        wpool = ctx.enter_context(tc.tile_pool(name="wpool", bufs=4))
        scrp = ctx.enter_context(tc.tile_pool(name="scrp", bufs=2))
        crfs = ctx.enter_context(tc.tile_pool(name="crfs", bufs=4))
        xTp = ctx.enter_context(tc.tile_pool(name="xT", bufs=1))

        # ---- persistent SBUF tiles ----
        ident_sb = consts.tile([128, 128], bf16, tag="ident")
        nc.sync.dma_start(ident_sb[:], ident[:])
        whh_sb = {}
        for l in range(2):
            for d in range(2):
                t_ = consts.tile([128, 512], bf16, name=f"whh{l}{d}")
                nc.sync.dma_start(t_[:], whh[l, d])
                whh_sb[l, d] = t_
        bias_sb = {}
        for l in range(2):
            for d in range(2):
                t_ = consts.tile([128, 4], f32, name=f"bias{l}{d}")
                nc.sync.dma_start(t_[:], biases[l, d])
                bias_sb[l, d] = t_
        wih0_sb = {}
        for d in range(2):
            for c in range(3):
                t_ = consts.tile([128, 512], bf16, name=f"wih0_{d}{c}")
                nc.sync.dma_start(t_[:], wih0[d, c])
                wih0_sb[d, c] = t_
        wih1_sb = {}
        for d in range(2):
            for k in range(2):
                t_ = consts.tile([128, 512], bf16, name=f"wih1_{d}{k}")
                nc.sync.dma_start(t_[:], wih1[d, k])
                wih1_sb[d, k] = t_
        woutT_sb = [consts.tile([128, L], bf16, name=f"wo{k}") for k in range(2)]
        for k in range(2):
            nc.sync.dma_start(woutT_sb[k][:], woutT[k])
        bout_sb = consts.tile([L, 1], f32, name="bout_sb")
        estart_sb = consts.tile([L, 1], f32, name="estart_sb")
        expE_sb = consts.tile([L, L], f32, name="expE_sb")
        expET_sb = consts.tile([L, L], f32, name="expET_sb")
        expend_sb = consts.tile([L, 1], f32, name="expend_sb")
        ones9 = consts.tile([L, L], f32, name="ones9")
        nc.sync.dma_start(bout_sb[:], bout[:])
        nc.sync.dma_start(estart_sb[:], estart[:])
        nc.sync.dma_start(expE_sb[:], expE[:])
        nc.sync.dma_start(expET_sb[:], expET[:])
        nc.sync.dma_start(expend_sb[:], expend[:])
        nc.vector.memset(ones9[:], 1.0)

        h_hist = {}
        for l in range(2):
            for d in range(2):
                h_hist[l, d] = states.tile([128, NT], bf16, name=f"h{l}{d}")
        # per-dir persistent cell tile, cols [o i f g C] (x BL each); the
        # C slot makes the fused (v2|z) DVE op's in1 = [g C] contiguous.
        ycell = [states.tile([128, 5 * BL], f32, name=f"yc{d}") for d in range(2)]
        tcl_st = [states.tile([128, BL], f32, name=f"tcl{d}") for d in range(2)]

        # xp for the two directions of the current layer (reused across layers)
        xp_sb = [states.tile([128, 4 * NT], bf16, name=f"xp{d}") for d in range(2)]

        # CRF persistent tiles
        expem = states.tile([L, NT], f32, name="expem")
        oh_sb = states.tile([L, NT], bf16, name="oh_sb")
        nc.sync.dma_start(oh_sb[:], oh[:])
        P_a = states.tile([L, BL], f32, name="P_a")       # alpha (SBUF leg)
        u_b = states.tile([L, BL], f32, name="u_b")       # beta (SBUF leg)
        lnacc = states.tile([1, BL], f32, name="lnacc")
        num9c = states.tile([L, NCK], f32, name="num9c")
        num9 = states.tile([L, 1], f32, name="num9")
        nc.vector.memset(lnacc[:], 0.0)

        def bulk_chunk(layer, d, nck, srcs):
            """xp[d][:, chunk nck] = srcs-matmul + bias, gate slots (o,i,f,g)."""
            nsl = slice(nck * NCOLS, (nck + 1) * NCOLS)
            xv = xp_sb[d][:].rearrange("p (t g b) -> p t g b", g=4, b=BL)
            tpc = NCOLS // BL
            for slot in range(4):
                pt = bulkp.tile([128, NCOLS], f32, tag="pt", name="pt")
                for ki, (src, wt, kk) in enumerate(srcs):
                    nc.tensor.matmul(
                        pt[:],
                        lhsT=wt[:kk, slot * 128:(slot + 1) * 128],
                        rhs=src[:kk, nsl],
                        start=(ki == 0), stop=(ki == len(srcs) - 1),
                    )
                nc.gpsimd.tensor_scalar(
                    out=xv[:, nck * tpc:(nck + 1) * tpc, slot, :],
                    in0=pt[:].rearrange("p (t b) -> p t b", b=BL),
                    scalar1=bias_sb[layer, d][:, slot:slot + 1],
                    scalar2=None, op0=ALU.add,
                )

        def recur_phase(layer):
            # Two independent dir-chains; each cell is PE -> ACT -> DVE(w)
            # -> DVE(C) -> ACT -> DVE(h). Wall time = T x chain path; the
            # two chains overlap on the engines.
            for d in range(2):
                nc.vector.memset(ycell[d][:, 4 * BL:5 * BL], 0.0)
            for t in range(Tsteps):
                taus = (t, Tsteps - 1 - t)
                first = (t == 0)
                for d in range(2):
                    tau = taus[d]
                    y = ycell[d]
                    gp = gatesp.tile([128, 4 * BL], f32, tag=f"gp{d}",
                                     name=f"gp{d}")
                    nc.tensor.matmul(gp[:], lhsT=ident_sb[:],
                                     rhs=xp_sb[d][:, tau * 4 * BL:(tau + 1) * 4 * BL],
                                     start=True, stop=first)
                    if not first:
                        prev = tau - 1 if d == 0 else tau + 1
                        hh = h_hist[layer, d]
                        whh_t = whh_sb[layer, d]
                        for slot in range(4):
                            nc.tensor.matmul(
                                gp[:, slot * BL:(slot + 1) * BL],
                                lhsT=whh_t[:, slot * 128:(slot + 1) * 128],
                                rhs=hh[:, prev * BL:(prev + 1) * BL],
                                start=False, stop=(slot == 3))
                    # y[0:4BL] = tanh(gates/2), slots (o,i,f,g)
                    nc.scalar.activation(y[:, 0:4 * BL], gp[:], ACTF.Tanh,
                                         scale=0.5)
                    # w = [(yi+1)*yg | (yf+1)*C_old] = [v2 | z]
                    w = wpool.tile([128, 2 * BL], f32, tag=f"w{d}", name=f"w{d}")
                    nc.vector.scalar_tensor_tensor(
                        w[:], in0=y[:, BL:3 * BL], scalar=1.0,
                        in1=y[:, 3 * BL:5 * BL], op0=ALU.add, op1=ALU.mult)
                    # C = 0.5*z + v2
                    nc.vector.scalar_tensor_tensor(
                        y[:, 4 * BL:5 * BL], in0=w[:, BL:2 * BL], scalar=0.5,
                        in1=w[:, 0:BL], op0=ALU.mult, op1=ALU.add)
                    nc.scalar.activation(tcl_st[d][:], y[:, 4 * BL:5 * BL],
                                         ACTF.Tanh, scale=0.5)
                    nc.vector.scalar_tensor_tensor(
                        h_hist[layer, d][:, tau * BL:(tau + 1) * BL],
                        in0=y[:, 0:BL], scalar=1.0, in1=tcl_st[d][:],
                        op0=ALU.add, op1=ALU.mult)

        # =================================================================
        # Phase 1: embedding gather + transpose (emitted from both sequence
        # ends so both bulk-0 dir-chunk streams start early)
        # =================================================================
        x_T = [xTp.tile([k, NT], bf16, name=f"xT{c}") for c, k in enumerate(KCH)]
        idx_all = consts.tile([128, NTILES], i32, name="idx_all")
        nc.sync.dma_start(idx_all[:], words[:])
        with tc.tile_pool(name="gath", bufs=4) as gp_:
            order = []
            lo, hi = 0, NTILES - 1
            while lo <= hi:
                order.append(lo)
                if hi != lo:
                    order.append(hi)
                lo, hi = lo + 1, hi - 1
            for i in order:
                g = gp_.tile([128, DPAD], bf16, tag="g", name="g")
                nc.gpsimd.indirect_dma_start(
                    out=g[:], out_offset=None, in_=emb[:],
                    in_offset=bass.IndirectOffsetOnAxis(ap=idx_all[:, i:i + 1],
                                                        axis=0),
                )
                # transpose on the (idle) PE + DVE copy-back: keeps the
                # HWDGE free so it never stalls the frozen PE stream.
                for c, k in enumerate(KCH):
                    tp = tpp.tile([128, 128], bf16, tag="tp", name="tp")
                    nc.tensor.transpose(tp[:], g[:, c * 128:(c + 1) * 128],
                                        ident_sb[:])
                    nc.vector.tensor_copy(out=x_T[c][:, i * 128:(i + 1) * 128],
                                          in_=tp[:])

        # bulk-0: fwd chunks ascending, bwd chunks descending, interleaved.
        # Deprioritized so the scheduler runs recurrence ops first in any
        # engine-idle gap (bulk fills the slack; data deps still hold).
        LOW = -10_000_000
        srcs0 = lambda d: [(x_T[c], wih0_sb[d, c], KCH[c]) for c in range(3)]
        with tc.high_priority(offset=LOW):
            for j in range(NCK):
                bulk_chunk(0, 0, j, srcs0(0))
                bulk_chunk(0, 1, NCK - 1 - j, srcs0(1))

        recur_phase(0)

        # bulk-1 middle-out: h0 regions complete middle-out during recur 0,
        # so these run concurrently with the tail of the layer-0 recurrence.
        srcs1 = lambda d: [(h_hist[0, k], wih1_sb[d, k], 128) for k in range(2)]
        mid_order = []
        lo, hi = NCK // 2 - 1, NCK // 2
        while lo >= 0:
            mid_order.extend([hi, lo])
            lo, hi = lo - 1, hi + 1
        with tc.high_priority(offset=LOW):
            for j in mid_order:
                for d in range(2):
                    bulk_chunk(1, d, j, srcs1(d))

        recur_phase(1)

        # =================================================================
        # Emissions (middle-out, overlap recur 1): per chunk
        #   pt = w_out @ h1 (PSUM); expem = exp(pt + bout') (ACT);
        #   num partial = sum(pt * onehot) via accum_out (DVE)
        # =================================================================
        with tc.high_priority(offset=LOW):
            for nck in mid_order:
                nsl = slice(nck * NCOLS, (nck + 1) * NCOLS)
                pt = bulkp.tile([L, NCOLS], f32, tag="pt", name="pt")
                for k in range(2):
                    nc.tensor.matmul(pt[:], lhsT=woutT_sb[k][:],
                                     rhs=h_hist[1, k][:, nsl],
                                     start=(k == 0), stop=(k == 1))
                nc.scalar.activation(expem[:, nsl], pt[:], ACTF.Exp,
                                     bias=bout_sb[:, 0:1])
                scr = scrp.tile([L, NCOLS], f32, tag="scr", name="scr")
                nc.vector.scalar_tensor_tensor(
                    scr[:], in0=pt[:], scalar=0.0, in1=oh_sb[:, nsl],
                    op0=ALU.add, op1=ALU.mult,
                    accum_out=num9c[:, nck:nck + 1])

        # =================================================================
        # CRF: two-ended scan in probability space.
        #   alpha: P_{t} = (E^T P_{t-1}) o expem_t   for t = 1..TM-1
        #   beta:  Q_{t} = E (expem_{t+1} o Q_{t+1}) for t = T-2..TM-1
        #   Z_b = sum_i alpha_{TM-1}[i] * beta_{TM-1}[i]
        # State ping-pongs PSUM (matmul) <-> SBUF (mult); one renorm each.
        # =================================================================
        def renorm(vec_sb, b_lnacc_col):
            srow = bulkp.tile([L, BL], f32, tag="pt", name="srow")
            nc.tensor.matmul(srow[:], lhsT=ones9[:], rhs=vec_sb[:],
                             start=True, stop=True)
            lns = crfs.tile([1, BL], f32, tag="lns", name="lns")
            nc.scalar.activation(lns[:], srow[0:1, :], ACTF.Ln)
            nc.vector.tensor_tensor(out=lnacc[:], in0=lnacc[:], in1=lns[:],
                                    op=ALU.add)
            rec = crfs.tile([L, BL], f32, tag="rec", name="rec")
            nc.vector.reciprocal(rec[:], srow[:])
            nc.vector.tensor_tensor(out=vec_sb[:], in0=vec_sb[:], in1=rec[:],
                                    op=ALU.mult)

        # alpha init: P_0 = exp(start) o expem_0
        nc.vector.tensor_scalar(out=P_a[:], in0=expem[:, 0:BL],
                                scalar1=estart_sb[:, 0:1], scalar2=None,
                                op0=ALU.mult)
        # beta init: u = expem_{T-1} o expend  (u for producing Q_{T-2})
        nc.vector.tensor_scalar(out=u_b[:], in0=expem[:, (Tsteps - 1) * BL:],
                                scalar1=expend_sb[:, 0:1], scalar2=None,
                                op0=ALU.mult)
        # chains: alpha consumes expem_1..expem_{TM-1};
        # beta consumes expem_{T-2}..expem_{TM} then combines at TM-1.
        n_a = TM - 1                 # alpha steps (t = 1..TM-1)
        n_b = Tsteps - TM            # beta matmul steps producing Q_{TM-1}
        for k in range(1, max(n_a, n_b) + 1):
            if k <= n_a:
                sp = crfp.tile([L, BL], f32, tag="tp", name="sp")
                nc.tensor.matmul(sp[:], lhsT=expE_sb[:], rhs=P_a[:],
                                 start=True, stop=True)
                nc.vector.tensor_tensor(
                    out=P_a[:], in0=sp[:],
                    in1=expem[:, k * BL:(k + 1) * BL], op=ALU.mult)
                if k == RENORM_K:
                    renorm(P_a, 0)
            if k <= n_b:
                sq = crfp.tile([L, BL], f32, tag="tp", name="sq")
                nc.tensor.matmul(sq[:], lhsT=expET_sb[:], rhs=u_b[:],
                                 start=True, stop=True)
                t_next = Tsteps - 1 - k      # Q_{t_next} now in sq
                if k < n_b:
                    nc.vector.tensor_tensor(
                        out=u_b[:], in0=sq[:],
                        in1=expem[:, t_next * BL:(t_next + 1) * BL],
                        op=ALU.mult)
                    if k == RENORM_K:
                        renorm(u_b, 0)
                else:
                    # final: u_b <- Q_{TM-1} (plain copy out of PSUM)
                    nc.vector.tensor_copy(out=u_b[:], in_=sq[:])

        # combine: Z_b = sum_i P_a[i,b] * u_b[i,b]; denom = ln Z + lnacc
        ab = crfs.tile([L, BL], f32, tag="ab", name="ab")
        nc.vector.tensor_tensor(out=ab[:], in0=P_a[:], in1=u_b[:], op=ALU.mult)
        zrow = bulkp.tile([L, BL], f32, tag="pt", name="zrow")
        nc.tensor.matmul(zrow[0:1, :], lhsT=ones9[:, 0:1], rhs=ab[:],
                         start=True, stop=True)
        lnz = crfs.tile([1, BL], f32, tag="lns", name="lnz")
        nc.scalar.activation(lnz[:], zrow[0:1, :], ACTF.Ln)
        nc.vector.tensor_tensor(out=lnz[:], in0=lnz[:], in1=lnacc[:],
                                op=ALU.add)
        dsc = crfs.tile([1, 1], f32, tag="dsc", name="dsc")
        nc.vector.tensor_reduce(dsc[:], lnz[:], axis=mybir.AxisListType.X,
                                op=ALU.add)
        nc.vector.tensor_reduce(num9[:], num9c[:], axis=mybir.AxisListType.X,
                                op=ALU.add)
        npsum = bulkp.tile([L, BL], f32, tag="pt", name="npsum")
        nc.tensor.matmul(npsum[0:1, 0:1], lhsT=ones9[:, 0:1], rhs=num9[:, 0:1],
                         start=True, stop=True)
        out_sb = crfs.tile([1, 2], f32, tag="out_sb", name="out_sb")
        nc.vector.tensor_scalar(out=out_sb[:, 0:1], in0=npsum[0:1, 0:1],
                                scalar1=0.0, scalar2=None, op0=ALU.add)
        nc.vector.tensor_scalar(out=out_sb[:, 1:2], in0=dsc[:],
                                scalar1=0.0, scalar2=None, op0=ALU.add)
        nc.sync.dma_start(res[:], out_sb[:])


# ---------------------------------------------------------------------------
# host side
# ---------------------------------------------------------------------------

def _prescale(w_ih, w_hh, b_ih, b_hh, h_in_doubled):
    """Gate-slot layout is torch order (i,f,g,o). Returns fp32 arrays."""
    sg = np.ones((4, 1), np.float32)
    sg[2] = 2.0                       # g gate rows x2 (tanh scale 0.5 trick)
    srows = np.repeat(sg, H, axis=0)  # [512, 1]
    wih = w_ih.astype(np.float32) * srows
    whh_ = w_hh.astype(np.float32) * srows * 0.5
    b = (b_ih + b_hh).astype(np.float32) * srows[:, 0]
    if h_in_doubled:
        wih = wih * 0.5
    return wih, whh_, b


GATE_ORDER = (3, 0, 1, 2)   # device slot s holds torch gate GATE_ORDER[s]: o,i,f,g


def _lhsT_gate_major(w, kchunks):
    """w: [4H, K] fp32 -> [nchunk, 128, 512] bf16 lhsT (zero-padded K)."""
    outs = []
    off = 0
    for kk in kchunks:
        blk = np.zeros((128, 512), np.float32)
        take = min(kk, w.shape[1] - off)
        for slot, g in enumerate(GATE_ORDER):
            blk[:take, slot * 128:(slot + 1) * 128] = \
                w[g * H:(g + 1) * H, off:off + take].T
        outs.append(blk)
        off += kk
    return np.stack(outs).astype(np.dtype("bfloat16"))


_PROG_CACHE = {}


def _get_program(Tsteps):
    if Tsteps not in _PROG_CACHE:
        _PROG_CACHE[Tsteps] = build_program(Tsteps)
    return _PROG_CACHE[Tsteps]


def prepare_inputs(inputs, Tsteps=T):
    """Build the per-core input maps + the host numerator constants."""
    bfl = np.dtype("bfloat16")
    words = np.asarray(inputs["word_batch"]).astype(np.int64)
    labels = np.asarray(inputs["label_batch"]).astype(np.int64)
    emb = np.asarray(inputs["emb"], np.float32)
    words = words[:, :Tsteps]
    labels = labels[:, :Tsteps]

    embp = np.zeros((V, DPAD), np.float32)
    embp[:, :D] = emb
    embp = embp.astype(bfl)

    ident = np.eye(128, dtype=np.float32).astype(bfl)

    wih0_l, whh_l, wih1_l, bias_l = [], [], [], []
    for layer, (wihk, whhk, bihk, bhhk) in enumerate(
            [("w_ih_l0", "w_hh_l0", "b_ih_l0", "b_hh_l0"),
             ("w_ih_l1", "w_hh_l1", "b_ih_l1", "b_hh_l1")]):
        for d in range(2):
            wih, whh_, b = _prescale(
                np.asarray(inputs[wihk])[d], np.asarray(inputs[whhk])[d],
                np.asarray(inputs[bihk])[d], np.asarray(inputs[bhhk])[d],
                h_in_doubled=(layer == 1))
            if layer == 0:
                wihp = np.zeros((512, DPAD), np.float32)
                wihp[:, :D] = wih
                wih0_l.append(_lhsT_gate_major(wihp, KCH))
            else:
                wih1_l.append(_lhsT_gate_major(wih, (128, 128)))
            whh_l.append(_lhsT_gate_major(whh_, (128,)))
            bias_l.append(b.reshape(4, H)[list(GATE_ORDER)].T)  # [128, 4]
    wih0 = np.stack(wih0_l)                       # [2, 3, 128, 512]
    wih1 = np.stack(wih1_l)                       # [2, 2, 128, 512]
    whh = np.stack(whh_l).reshape(2, 2, 1, 128, 512)[:, :, 0]
    biases = np.stack(bias_l).reshape(2, 2, 128, 4).astype(np.float32)

    w_out = np.asarray(inputs["w_out"], np.float32) * 0.5   # [L, 2H]
    woutT = np.stack([w_out[:, :H].T, w_out[:, H:].T]).astype(bfl)  # [2,128,L]
    boutp = (np.asarray(inputs["b_out"], np.float32) - CBAR).reshape(L, 1)
    estart = np.exp(np.asarray(inputs["start_t"], np.float32)).reshape(L, 1)
    expEm = np.exp(np.asarray(inputs["trans"], np.float32))
    expend = np.exp(np.asarray(inputs["end_t"], np.float32)).reshape(L, 1)

    NT = Tsteps * BL
    in_maps = []
    num_consts = []
    start_t = np.asarray(inputs["start_t"], np.float32)
    end_t = np.asarray(inputs["end_t"], np.float32)
    trans = np.asarray(inputs["trans"], np.float32)
    bshift = np.asarray(inputs["b_out"], np.float32) - CBAR   # device em offset
    for c in range(NCORES):
        bs = slice(c * BL, (c + 1) * BL)
        wc = words[bs]                        # [BL, Tsteps]
        lc = labels[bs]
        toks = wc.T.reshape(-1).astype(np.int32)          # (t, b) order
        ohc = (lc.T.reshape(1, -1) == np.arange(L).reshape(L, 1))
        in_maps.append({
            "words": np.ascontiguousarray(toks.reshape(NT // 128, 128).T),
            "emb": embp, "ident": ident,
            "wih0": wih0, "wih1": wih1, "whh": whh, "biases": biases,
            "woutT": woutT, "bout": boutp, "estart": estart,
            "expE": expEm, "expET": np.ascontiguousarray(expEm.T),
            "expend": expend,
            "oh": ohc.astype(bfl),
        })
        num_consts.append(
            float(start_t[lc[:, 0]].sum())
            + float(trans[lc[:, :-1], lc[:, 1:]].sum())
            + float(end_t[lc[:, -1]].sum())
            + float(bshift[lc].sum()))        # device num omits the em bias
    return in_maps, num_consts


def kernel(**inputs):
    in_maps, num_consts = prepare_inputs(inputs, T)
    nc = _get_program(T)
    out = run_bass_kernel_spmd(nc, in_maps, list(range(NCORES)))
    llh = 0.0
    for c in range(NCORES):
        r = out.results[c]["res"].reshape(2).astype(np.float64)
        llh += num_consts[c] + r[0] - r[1]
    return np.float32(-llh)


if __name__ == "__main__":
    np.random.seed(0)
    print("building program (small T) ...")
    build_program(16)
    print("ok")
